# revision 18
# baseline (speedup 1.0000x reference)
"""Trainium2 Bass kernel for SSD MultiBox loss (nn_ModelLoss_5970004541458).

Strategy: data-parallel over batch (32 images -> 8 cores x 4 images).
Per core, everything over the prior dim (P=8732, padded to 8960 = 70*128)
runs on-device:
  - jaccard matching in bf16 log-IoU space (monotone, so max/argmax/threshold
    comparisons are unchanged; threshold ln 0.5). Paired (x,y) ops halve the
    instruction count; bf16 doubles DVE throughput.
  - forced assignment via ADDITIVE sentinels ov + fmask*(100+4k): the 4-unit
    k spacing exceeds the ln-IoU range of forced points, so the largest k
    wins among colliding boxes (emulates the reference's last-wins scatter).
  - per-prior one-hot box gather via PE transpose + block-diag matmul (bf16)
  - CE: exp on ACT (bf16, 2 big chunks), class-sums on DVE (bf16),
    score-at-label via PE with sres stationary (81-col LDW, 16-col moving)
    and a CPU-precomputed label one-hot.
  - hard-negative mining via a 2-level 16-way counting grid with bounded-error
    boundary correction (no sort), per image.
All DRAM inputs are laid out per-partition-contiguous so every load is one
large DMA (128 descriptors of >=512B): one const pack, one image pack
(locs+boxes+labels-one-hot, CPU-pre-broadcast), one scores+qblk DMA per image.
Each core returns 16 partial sums; the host combines them into the loss.
"""
import sys

for _p in ("/opt/trn_rl_repo",):
    if _p not in sys.path:
        sys.path.insert(0, _p)

import numpy as np

import concourse.bass as bass
import concourse.tile as tile
from concourse import mybir
from concourse.bass_utils import run_bass_kernel_spmd

F32 = mybir.dt.float32
BF16 = mybir.dt.bfloat16
AX = mybir.AxisListType
OP = mybir.AluOpType
ACTF = mybir.ActivationFunctionType

B, P, C, K = 32, 8732, 81, 16
NCORES = 8
I = B // NCORES          # images per core = 4
PP = 8960                # padded priors = 70 * 128
T = PP // 128            # 70 prior tiles
T2 = 72                  # padded tile count for 128-col transpose blocks
NB = T2 * K // 128       # 9 transpose blocks of 128 (t,k)-columns
NCH = 2                  # score chunks per image (35 tiles each)
CT = T // NCH            # tiles per chunk = 35
THRESHOLD = 0.5
LN_THR = float(np.log(0.5))  # positives threshold in log-IoU space
KV0 = 100.0              # forced-assignment sentinel base (added to ln-IoU)
KVS = 4.0                # sentinel k spacing (> ln-IoU range of forced points)
NQ = 5                   # gathered quantities per box (cx, cy, 5lnw, 5lnh, pad)

# const pack column offsets (f32, [128, CW])
CO_PT = 0                # 11 prior-table rows x 70
CO_IDF = 770             # f32 identity 128
CO_IO15 = 898            # (1..15)/16 then 999
CO_KV16 = 914            # 100 + 4k
CO_THR = 930             # 0..15 level-1 mining thresholds
CW = 946

# image pack column offsets (f32, [128, I, IW])
IO_LOC = 0               # T2*4 locs (t-major, tail tiles zero)
IO_BB = 288              # 5x16 box rows (x1,y1,x2,y2,area), broadcast on CPU
IO_LM = 368              # label one-hot [81, 16] on partitions 0..80
IW = 384

# scores pack (bf16, [I, 128, SW])
SO_SC = 0                # 70*81 scores (t-major)
SO_QB = 5670             # 8*NQ block-diag gather stationary
SW = 5670 + 8 * NQ

_bf16 = np.dtype("uint16")  # bf16 carried as uint16 bit pattern if ml_dtypes absent
try:
    import ml_dtypes

    _bf16 = np.dtype(ml_dtypes.bfloat16)
except ImportError:
    ml_dtypes = None


def _to_bf16(x: np.ndarray) -> np.ndarray:
    if ml_dtypes is not None:
        return x.astype(ml_dtypes.bfloat16)
    u = x.astype(np.float32).view(np.uint32)
    rounded = ((u >> 16) + ((u >> 15) & 1)).astype(np.uint32)
    return (rounded & 0xFFFF).astype(np.uint16)


def _fixup_module(nc: bass.Bass) -> None:
    """Adapt the Tile-generated module to this container's walrus build.

    - EVENT_SEMAPHORE_RANGE_CLEAR is rejected ("ISA wrong length"); the
      preceding Drain(is_reset_sema) already resets the same range, so drop it.
    - Seq-only instructions accept fewer sync waits than Tile emits; hoist
      excess waits onto NoOps placed immediately before (same engine, so
      program order preserves semantics).
    """
    import bass_rust

    for f in nc.m.functions:
        for blk in f.blocks:
            newl = []
            for ins in blk.instructions:
                if getattr(ins, "op_name", None) == "EVENT_SEMAPHORE_RANGE_CLEAR":
                    continue
                si = ins.sync_info
                maxw = 1
                if si is not None and si.on_wait and len(si.on_wait) > maxw:
                    waits = list(si.on_wait)
                    extra, keep = waits[:-maxw], waits[-maxw:]
                    for j in range(0, len(extra), 1):
                        nop = mybir.InstNoOp(
                            name=f"{ins.name}-wsplit{j}", ins=[], outs=[],
                            engine=ins.engine)
                        nop.sync_info = bass_rust.SyncInfo(
                            on_wait=[extra[j]], on_update=[])
                        newl.append(nop)
                    ins.sync_info = bass_rust.SyncInfo(
                        on_wait=keep,
                        on_update=list(si.on_update) if si.on_update else [])
                newl.append(ins)
            blk.instructions = newl


def build_nc(fixup: bool = True) -> bass.Bass:
    nc = bass.Bass()

    d_sco = nc.dram_tensor("sco", [I, 128, SW], BF16, kind="ExternalInput")
    d_ipack = nc.dram_tensor("ipack", [128, I * IW], F32, kind="ExternalInput")
    d_cst = nc.dram_tensor("cst", [128, CW], F32, kind="ExternalInput")
    d_identb = nc.dram_tensor("identb", [128, 128], BF16, kind="ExternalInput")
    # out row layout (single partition): [np0..3, box0..3, cep0..3, mine0..3]
    d_out = nc.dram_tensor("out", [1, 16], F32, kind="ExternalOutput")

    from contextlib import ExitStack

    with tile.TileContext(nc) as tc, ExitStack() as es:
        cpool = es.enter_context(tc.tile_pool(name="consts", bufs=1))
        spool = es.enter_context(tc.tile_pool(name="scores", bufs=2))
        wpool = es.enter_context(tc.tile_pool(name="work", bufs=2))
        epool = es.enter_context(tc.tile_pool(name="exp", bufs=3))
        bpool = es.enter_context(tc.tile_pool(name="batched", bufs=1))
        pp_t = es.enter_context(tc.tile_pool(name="ps_t", bufs=1, space="PSUM"))
        pp_sel = es.enter_context(tc.tile_pool(name="ps_sel", bufs=1, space="PSUM"))
        pp_u = es.enter_context(tc.tile_pool(name="ps_u", bufs=1, space="PSUM"))
        pp_r = es.enter_context(tc.tile_pool(name="ps_r", bufs=1, space="PSUM"))

        # ---------------- constants (3 DMAs total) ----------------
        cpack = cpool.tile([128, CW], F32, tag="cpack")
        nc.sync.dma_start(out=cpack[:], in_=d_cst[:, :])
        ident = cpool.tile([128, 128], BF16, tag="ident")
        nc.sync.dma_start(out=ident[:], in_=d_identb[:, :])
        ipk = cpool.tile([128, I, IW], F32, tag="ipk")
        nc.sync.dma_start(out=ipk[:].rearrange("p i w -> p (i w)"),
                          in_=d_ipack[:, :])

        names = ["px1", "py1", "px2", "py2", "parea", "pcxn", "pcyn",
                 "ivx10", "ivy10", "lpw5", "lph5"]
        pt = {nm: cpack[:, CO_PT + r * T:CO_PT + (r + 1) * T]
              for r, nm in enumerate(names)}
        identf = cpack[:, CO_IDF:CO_IDF + 128]
        io15 = cpack[:, CO_IO15:CO_IO15 + 16]
        thrL1 = cpack[:, CO_THR:CO_THR + 16]

        ones_p = cpool.tile([128, 1], F32, tag="ones_p")
        nc.vector.memset(ones_p[:], 1.0)
        ones_r = cpool.tile([1, 128], F32, tag="ones_r")
        nc.vector.memset(ones_r[:], 1.0)
        ones_rb = cpool.tile([1, 128], BF16, tag="ones_rb")
        nc.vector.memset(ones_rb[:], 1.0)
        eps_b = cpool.tile([128, 1], F32, tag="eps_b")
        nc.vector.memset(eps_b[:], 1e-20)

        # bf16 staging copies of jaccard constants
        pt12b = cpool.tile([128, 2, T], BF16, tag="pt12b")
        nc.scalar.copy(pt12b[:].rearrange("p r t -> p (r t)"),
                       cpack[:, CO_PT:CO_PT + 2 * T])
        pt34b = cpool.tile([128, 2, T], BF16, tag="pt34b")
        nc.scalar.copy(pt34b[:].rearrange("p r t -> p (r t)"),
                       cpack[:, CO_PT + 2 * T:CO_PT + 4 * T])
        pareab = cpool.tile([128, T], BF16, tag="pareab")
        nc.scalar.copy(pareab[:], pt["parea"])
        kv16b = cpool.tile([128, K], BF16, tag="kv16b")
        nc.scalar.copy(kv16b[:], cpack[:, CO_KV16:CO_KV16 + K])

        def rowsum(dst_row_ap, src_ap, n):
            """[P, n] f32 -> [1, n] partition sum written to dst_row_ap."""
            ps = pp_r.tile([1, 128], F32, tag="red_row")
            nc.tensor.matmul(ps[:, :n], lhsT=ones_p[:src_ap.shape[0], :],
                             rhs=src_ap, start=True, stop=True)
            nc.scalar.copy(dst_row_ap, ps[:, :n])

        def bcast_row(dst_ap, row_ap, n, bf=False):
            """[1, n] -> [128, n] replicated."""
            ps = pp_r.tile([128, 128], F32, tag="red_bc")
            nc.tensor.matmul(ps[:, :n], lhsT=ones_rb[:] if bf else ones_r[:],
                             rhs=row_ap, start=True, stop=True)
            nc.scalar.copy(dst_ap, ps[:, :n])

        def maxreduce_row(dst_row_ap, src_ap, n):
            """[128, n] f32 -> [1, n] partition max written to dst_row_ap."""
            ps = pp_r.tile([128, 128], F32, tag="red_bc")
            nc.tensor.transpose(ps[:n, :], src_ap, identf)
            tsb = cpool.tile([128, 128], F32, tag="red_tsb")
            nc.scalar.copy(tsb[:n, :], ps[:n, :])
            mx = cpool.tile([128, 1], F32, tag="red_mx")
            nc.vector.tensor_reduce(out=mx[:n, :], in_=tsb[:n, :],
                                    axis=AX.X, op=OP.max)
            ps2 = pp_r.tile([1, 128], F32, tag="red_row")
            nc.tensor.transpose(ps2[:, :n], mx[:n, :], identf[:n, :n])
            nc.scalar.copy(dst_row_ap, ps2[:, :n])

        # IV4/PC4: [128, T2, 4] with d = (x, y, w, h); tail t>=T zeroed
        iv4 = cpool.tile([128, T2, 4], F32, tag="iv4")
        pc4 = cpool.tile([128, T2, 4], F32, tag="pc4")
        nc.vector.memset(iv4[:], 0.0)
        nc.vector.memset(pc4[:], 0.0)
        nc.vector.tensor_copy(iv4[:, :T, 0], pt["ivx10"])
        nc.vector.tensor_copy(iv4[:, :T, 1], pt["ivy10"])
        nc.vector.memset(iv4[:, :T, 2], 1.0)
        nc.vector.memset(iv4[:, :T, 3], 1.0)
        nc.vector.tensor_copy(pc4[:, :T, 0], pt["pcxn"])
        nc.vector.tensor_copy(pc4[:, :T, 1], pt["pcyn"])
        nc.vector.tensor_copy(pc4[:, :T, 2], pt["lpw5"])
        nc.vector.tensor_copy(pc4[:, :T, 3], pt["lph5"])

        # persistent accumulators
        nprow = bpool.tile([1, I], F32, tag="nprow")
        scadd = bpool.tile([128, I, 4], F32, tag="scadd")   # fs, cn, lps, box
        scrow = bpool.tile([1, I, 4], F32, tag="scrow")
        bm4 = bpool.tile([128, I], F32, tag="bm4")
        bmrow = bpool.tile([1, I], F32, tag="bmrow")
        ufall = bpool.tile([C, I], F32, tag="ufall")
        uf4 = bpool.tile([1, I], F32, tag="uf4")
        out_sb = bpool.tile([1, 16], F32, tag="out_sb")

        for i in range(I):
            # ---------------- per-image load (1 DMA) ----------------
            sct = spool.tile([128, SW], BF16, tag="sct")
            nc.sync.dma_start(out=sct[:], in_=d_sco[i, :, :])
            sres = sct[:, :SO_QB].rearrange("p (t c) -> p t c", c=C)
            qblk = sct[:, SO_QB:SW]
            l4 = ipk[:, i, IO_LOC:IO_LOC + T2 * 4].rearrange(
                "p (t d) -> p t d", d=4)
            lmv = ipk[0:C, i, IO_LM:IO_LM + K]
            bbb = wpool.tile([128, 5, K], BF16, tag="bbb")
            nc.scalar.copy(bbb[:].rearrange("p a k -> p (a k)"),
                           ipk[:, i, IO_BB:IO_BB + 5 * K])

            # ---------------- jaccard, paired (x,y) in bf16 ----------------
            lt2 = wpool.tile([128, 2, T, K], BF16, tag="lt2")
            wh2 = wpool.tile([128, 2, T, K], BF16, tag="wh2")
            iu2 = wpool.tile([128, 2, T, K], BF16, tag="iu2")
            lnb = wpool.tile([128, 2, T, K], BF16, tag="lnb")
            ov = wpool.tile([128, T, K], BF16, tag="ov")
            nc.vector.tensor_tensor(
                out=lt2[:],
                in0=pt12b[:][:, :, :, None].broadcast_to([128, 2, T, K]),
                in1=bbb[:, 0:2, :][:, :, None, :].broadcast_to([128, 2, T, K]),
                op=OP.max)
            nc.vector.tensor_tensor(
                out=wh2[:],
                in0=pt34b[:][:, :, :, None].broadcast_to([128, 2, T, K]),
                in1=bbb[:, 2:4, :][:, :, None, :].broadcast_to([128, 2, T, K]),
                op=OP.min)
            nc.vector.tensor_sub(wh2[:], wh2[:], lt2[:])
            nc.scalar.activation(wh2[:], wh2[:], ACTF.Relu)
            nc.vector.tensor_mul(iu2[:, 0], wh2[:, 0], wh2[:, 1])
            nc.vector.tensor_tensor(
                out=iu2[:, 1],
                in0=pareab[:][:, :, None].broadcast_to([128, T, K]),
                in1=bbb[:, 4, :][:, None, :].broadcast_to([128, T, K]),
                op=OP.add)
            nc.vector.tensor_sub(iu2[:, 1], iu2[:, 1], iu2[:, 0])
            # log-space IoU: monotone, so comparisons unchanged
            nc.scalar.activation(lnb[:], iu2[:], ACTF.Ln, bias=eps_b[:])
            nc.vector.tensor_sub(ov[:], lnb[:, 0], lnb[:, 1])

            # ---------------- matching pass 2 ----------------
            m16 = wpool.tile([128, K], F32, tag="m16")
            m16r = wpool.tile([128, K], F32, tag="m16r")
            nc.vector.tensor_reduce(
                out=m16[:], in_=ov[:].rearrange("p t k -> p k t"),
                axis=AX.X, op=OP.max)
            m16row = wpool.tile([1, K], F32, tag="m16row")
            maxreduce_row(m16row[:], m16[:], K)
            bcast_row(m16r[:], m16row[:], K)
            fmask = wpool.tile([128, T, K], BF16, tag="fmask")
            nc.vector.tensor_tensor(
                out=fmask[:], in0=ov[:],
                in1=m16r[:][:, None, :].broadcast_to([128, T, K]),
                op=OP.is_equal)
            ovf = wpool.tile([128, T, K], BF16, tag="ovf")
            nc.vector.tensor_tensor(
                out=fmask[:], in0=fmask[:],
                in1=kv16b[:][:, None, :].broadcast_to([128, T, K]), op=OP.mult)
            nc.vector.tensor_add(ovf[:], ov[:], fmask[:])
            pm = wpool.tile([128, T], BF16, tag="pm")
            nc.vector.tensor_reduce(out=pm[:], in_=ovf[:], axis=AX.X, op=OP.max)
            ohb = wpool.tile([128, T2 * K], BF16, tag="ohb")
            nc.vector.memset(ohb[:, T * K:], 0.0)
            nc.vector.tensor_tensor(
                out=ohb[:, :T * K].rearrange("p (t k) -> p t k", k=K),
                in0=ovf[:],
                in1=pm[:][:, :, None].broadcast_to([128, T, K]),
                op=OP.is_equal)
            pos = wpool.tile([128, T], F32, tag="pos")
            nc.vector.tensor_scalar(out=pos[:], in0=pm[:],
                                    scalar1=LN_THR, scalar2=None,
                                    op0=OP.is_ge)
            wmat = wpool.tile([128, T, K], BF16, tag="wmat")
            nc.vector.tensor_tensor(
                out=wmat[:],
                in0=ohb[:, :T * K].rearrange("p (t k) -> p t k", k=K),
                in1=pos[:][:, :, None].broadcast_to([128, T, K]),
                op=OP.mult)

            # n_pos for this image
            npt = wpool.tile([128, 1], F32, tag="npt")
            nc.vector.tensor_scalar(out=pos[:], in0=pos[:],
                                    scalar1=1.0, scalar2=None, op0=OP.mult,
                                    op1=OP.add, accum_out=npt[:])
            rowsum(nprow[:, i:i + 1], npt[:], 1)
            npb = wpool.tile([128, 1], F32, tag="npb")
            bcast_row(npb[:], nprow[:, i:i + 1], 1)
            k3b = wpool.tile([128, 1], F32, tag="k3b")
            nc.gpsimd.tensor_scalar(out=k3b[:], in0=npb[:], scalar1=3.0,
                                    scalar2=None, op0=OP.mult)

            # ---------------- box gather via PE ----------------
            ohT_ps = pp_t.tile([128, NB, 128], BF16, tag="ohT")
            for b in range(NB):
                nc.tensor.transpose(
                    ohT_ps[:, b, :],
                    ohb[:, b * 128:(b + 1) * 128],
                    ident[:])
            ohT_sb = wpool.tile([128, NB * 128], BF16, tag="ohT_sb")
            nc.scalar.copy(ohT_sb[:], ohT_ps[:].rearrange("p b n -> p (b n)"))

            sel_ps = pp_sel.tile([8 * NQ, NB, 128], F32, tag="sel")
            for b in range(NB):
                nc.tensor.matmul(sel_ps[:, b, :], lhsT=qblk[:],
                                 rhs=ohT_sb[:, b * 128:(b + 1) * 128],
                                 start=True, stop=True)
            sel_sb = wpool.tile([8 * NQ, NB * 128], BF16, tag="sel_sb")
            nc.scalar.copy(sel_sb[:], sel_ps[:].rearrange("p b n -> p (b n)"))
            bk_ps = pp_t.tile([128, NB, 8 * NQ], BF16, tag="ohT")
            for b in range(NB):
                nc.tensor.transpose(
                    bk_ps[:, b, :],
                    sel_sb[:, b * 128:(b + 1) * 128],
                    ident[:8 * NQ, :8 * NQ])
            selq = wpool.tile([128, NB * 8 * NQ], F32, tag="selq")
            nc.scalar.copy(selq[:], bk_ps[:].rearrange("p b n -> p (b n)"))
            # selq[p, (blk*40 + tb*5 + q)] = sel_q at t = blk*8+tb
            sel4 = selq[:].rearrange("p (t q) -> p t q", q=NQ)[:, :, 0:4]

            # ---------------- box L1 ----------------
            lp4 = wpool.tile([128, T2, 4], F32, tag="lp4")
            nc.vector.tensor_add(lp4[:], l4, pc4[:])
            tb1 = wpool.tile([128, T2, 4], F32, tag="tb1")
            nc.vector.tensor_mul(tb1[:], sel4, iv4[:])
            nc.vector.tensor_sub(tb1[:], lp4[:], tb1[:])
            nc.vector.tensor_tensor(
                out=tb1[:, :T, :], in0=tb1[:, :T, :],
                in1=pos[:][:, :, None].broadcast_to([128, T, 4]),
                op=OP.mult)
            bacc = wpool.tile([128, 1], F32, tag="bacc")
            nc.scalar.activation(tb1[:], tb1[:], ACTF.Abs, accum_out=bacc[:])
            nc.scalar.copy(scadd[:, i, 3:4], bacc[:])

            # ------------- score at label: sres stationary on PE -------------
            u_ps = pp_u.tile([C, K], F32, tag="u")
            for t_ in range(T):
                nc.tensor.matmul(u_ps[:], lhsT=sres[:, t_, :],
                                 rhs=wmat[:, t_, :],
                                 start=(t_ == 0), stop=(t_ == T - 1))
            u_sb = wpool.tile([C, K], F32, tag="u_sb")
            nc.scalar.copy(u_sb[:], u_ps[:])
            ufx = wpool.tile([C, K], F32, tag="ufx")
            ufa = wpool.tile([C, 1], F32, tag="ufa")
            nc.vector.tensor_mul(ufx[:], u_sb[:], lmv)
            nc.vector.tensor_scalar(out=ufx[:], in0=ufx[:], scalar1=1.0,
                                    scalar2=None, op0=OP.mult, op1=OP.add,
                                    accum_out=ufa[:])
            nc.scalar.copy(ufall[:, i:i + 1], ufa[:])

            # ---------------- CE: exp on ACT + DVE reduces ----------------
            se = wpool.tile([128, T], BF16, tag="se")
            for ch in range(NCH):
                et = epool.tile([128, CT, C], BF16, tag="exps")
                nc.scalar.activation(
                    et[:], sres[:, ch * CT:(ch + 1) * CT, :], ACTF.Exp)
                with nc.allow_low_precision("bf16 lse; 2e-2 loss tolerance"):
                    nc.vector.tensor_reduce(
                        out=se[:, ch * CT:(ch + 1) * CT],
                        in_=et[:], axis=AX.X, op=OP.add)

            lse = wpool.tile([128, T], F32, tag="lse")
            nc.scalar.activation(lse[:], se[:], ACTF.Ln)
            ce0 = wpool.tile([128, T], F32, tag="ce0")
            nc.vector.tensor_sub(ce0[:], lse[:], sres[:, :, 0])
            cen = wpool.tile([128, T], F32, tag="cen")
            nc.vector.scalar_tensor_tensor(
                out=cen[:], in0=pos[:], scalar=THRESHOLD, in1=ce0[:],
                op0=OP.is_lt, op1=OP.mult)
            # ce_pos partial: sum(lse * pos) (minus U part in final combine)
            lpst = wpool.tile([128, T], F32, tag="lpst")
            lps = wpool.tile([128, 1], F32, tag="lps")
            nc.vector.scalar_tensor_tensor(
                out=lpst[:], in0=pos[:], scalar=1.0, in1=lse[:],
                op0=OP.mult, op1=OP.mult, accum_out=lps[:])
            nc.scalar.copy(scadd[:, i, 2:3], lps[:])

            # ---------------- mining (2-level 16-way grid) ----------------
            msk = wpool.tile([128, 16, T], F32, tag="msk")
            cnt16 = wpool.tile([128, 16], F32, tag="cnt16")
            nc.vector.tensor_tensor(
                out=msk[:],
                in0=cen[:][:, None, :].broadcast_to([128, 16, T]),
                in1=thrL1[:, :, None].broadcast_to([128, 16, T]),
                op=OP.is_gt)
            nc.vector.tensor_reduce(out=cnt16[:], in_=msk[:], axis=AX.X,
                                    op=OP.add)
            c1row = wpool.tile([1, 16], F32, tag="c1row")
            rowsum(c1row[:], cnt16[:], 16)
            cntr16 = wpool.tile([128, 16], F32, tag="cntr16")
            bcast_row(cntr16[:], c1row[:], 16)
            # lo = (#edges with count >= k) - 1   (edges j = 0..15)
            ge16 = wpool.tile([128, 16], F32, tag="ge16")
            lo1 = wpool.tile([128, 1], F32, tag="lo1")
            nc.vector.tensor_scalar(out=ge16[:], in0=cntr16[:],
                                    scalar1=k3b[:], scalar2=None,
                                    op0=OP.is_ge, op1=OP.add,
                                    accum_out=lo1[:])
            nc.vector.tensor_scalar(out=lo1[:], in0=lo1[:], scalar1=-1.0,
                                    scalar2=None, op0=OP.add)
            lop1 = wpool.tile([128, 1], F32, tag="lop1")
            nc.gpsimd.tensor_scalar(out=lop1[:], in0=lo1[:], scalar1=1.0 / 16,
                                    scalar2=None, op0=OP.add)
            # level 2: thresholds lo + m/16 (io15 has (1..15)/16 then +999)
            thr2 = wpool.tile([128, 16], F32, tag="thr2")
            nc.vector.tensor_scalar(out=thr2[:], in0=io15,
                                    scalar1=lo1[:], scalar2=None,
                                    op0=OP.add)
            msc2 = wpool.tile([128, 16, T], F32, tag="msc2")
            c2 = wpool.tile([128, 16], F32, tag="c2")
            nc.vector.tensor_tensor(
                out=msc2[:],
                in0=cen[:][:, None, :].broadcast_to([128, 16, T]),
                in1=thr2[:][:, :, None].broadcast_to([128, 16, T]),
                op=OP.is_gt)
            nc.vector.tensor_reduce(out=c2[:], in_=msc2[:], axis=AX.X,
                                    op=OP.add)
            c2row = wpool.tile([1, 16], F32, tag="c2row")
            rowsum(c2row[:], c2[:], 16)
            c2r = wpool.tile([128, 16], F32, tag="c2r")
            bcast_row(c2r[:], c2row[:], 16)
            ge2 = wpool.tile([128, 16], F32, tag="ge2")
            mc = wpool.tile([128, 1], F32, tag="mc")
            nc.vector.tensor_scalar(out=ge2[:], in0=c2r[:],
                                    scalar1=k3b[:], scalar2=None,
                                    op0=OP.is_ge, op1=OP.add, accum_out=mc[:])
            hi1 = wpool.tile([128, 1], F32, tag="hi1")
            nc.vector.tensor_scalar(out=hi1[:], in0=mc[:],
                                    scalar1=1.0 / 16, scalar2=lop1[:],
                                    op0=OP.mult, op1=OP.add)
            # F(hi), count(hi), boundary max
            fsc = wpool.tile([128, T], F32, tag="fsc")
            fsa = wpool.tile([128, 1], F32, tag="fsa")
            nc.vector.scalar_tensor_tensor(
                out=fsc[:], in0=cen[:], scalar=hi1[:],
                in1=cen[:], op0=OP.is_gt, op1=OP.mult,
                accum_out=fsa[:])
            nc.scalar.copy(scadd[:, i, 0:1], fsa[:])
            cna = wpool.tile([128, 1], F32, tag="cna")
            nc.vector.tensor_scalar(out=fsc[:], in0=cen[:],
                                    scalar1=hi1[:], scalar2=None,
                                    op0=OP.is_gt, op1=OP.add, accum_out=cna[:])
            nc.scalar.copy(scadd[:, i, 1:2], cna[:])
            nc.vector.scalar_tensor_tensor(
                out=fsc[:], in0=cen[:], scalar=hi1[:],
                in1=cen[:], op0=OP.is_le, op1=OP.mult)
            bmt = wpool.tile([128, 1], F32, tag="bmt")
            nc.vector.tensor_reduce(out=bmt[:], in_=fsc[:], axis=AX.X, op=OP.max)
            nc.scalar.copy(bm4[:, i:i + 1], bmt[:])

        # ---------------- final combine (partition 0) ----------------
        rowsum(scrow[:].rearrange("p i s -> p (i s)"),
               scadd[:].rearrange("p i s -> p (i s)"), I * 4)
        maxreduce_row(bmrow[:], bm4[:], I)
        rowsum(uf4[:], ufall[:], I)

        k34r = bpool.tile([1, I], F32, tag="k34r")
        nc.vector.tensor_scalar(out=k34r[:], in0=nprow[:], scalar1=3.0,
                                scalar2=None, op0=OP.mult)
        r4 = bpool.tile([1, I], F32, tag="r4")
        nc.vector.tensor_sub(r4[:], k34r[:], scrow[:, :, 1])
        nc.vector.tensor_mul(r4[:], r4[:], bmrow[:])
        nc.vector.tensor_add(r4[:], r4[:], scrow[:, :, 0])   # mine sums
        cep = bpool.tile([1, I], F32, tag="cep")
        nc.vector.tensor_sub(cep[:], scrow[:, :, 2], uf4[:])  # ce_pos sums
        nc.vector.tensor_copy(out_sb[:, 0:4], nprow[:])
        nc.vector.tensor_copy(out_sb[:, 4:8], scrow[:, :, 3])
        nc.vector.tensor_copy(out_sb[:, 8:12], cep[:])
        nc.vector.tensor_copy(out_sb[:, 12:16], r4[:])
        nc.sync.dma_start(out=d_out[:, :], in_=out_sb[:])

    if fixup:
        _fixup_module(nc)
    return nc


def prepare_inputs(predicted_locs, predicted_scores, boxes, labels,
                   priors_centers):
    """Shard + marshal the full inputs into 8 per-core in_maps.

    All DRAM layouts are per-partition contiguous (partition-major), so
    every SBUF partition reads one contiguous chunk per DMA.
    """
    predicted_locs = np.asarray(predicted_locs, np.float32)
    predicted_scores = np.asarray(predicted_scores, np.float32)
    boxes = np.asarray(boxes, np.float32)
    labels_f = np.asarray(labels).astype(np.int64)
    priors = np.asarray(priors_centers, np.float32)

    npad = PP - P
    # scores: pad rows have class0=0, others -50 -> lse=0, S0=0, ce0=0 exactly
    pad_scores = np.full((B, npad, C), -50.0, np.float32)
    pad_scores[:, :, 0] = 0.0
    scores_p = np.concatenate([predicted_scores, pad_scores], axis=1)
    # [B, PP, C] -> [B, T, 128, C] -> [B, 128, T, C] -> [B, 128, T*C]
    scores_pm = scores_p.reshape(B, T, 128, C).transpose(0, 2, 1, 3)

    bx1, by1, bx2, by2 = (boxes[:, :, d] for d in range(4))
    barea = (bx2 - bx1) * (by2 - by1)
    q5 = np.stack([
        (bx1 + bx2) / 2, (by1 + by2) / 2,
        5.0 * np.log(bx2 - bx1), 5.0 * np.log(by2 - by1),
        np.zeros_like(bx1),
    ], axis=2).astype(np.float32)                           # [B, K, 5]
    qblk = np.zeros((B, 128, 8 * NQ), np.float32)
    for tb in range(8):
        qblk[:, tb * K:(tb + 1) * K, tb * NQ:(tb + 1) * NQ] = q5

    sco = np.zeros((B, 128, SW), np.float32)
    sco[:, :, :SO_QB] = scores_pm.reshape(B, 128, T * C)
    sco[:, :, SO_QB:] = qblk
    sco = _to_bf16(sco)

    # image pack: locs (t-major, tail zero) + broadcast box rows + label 1-hot
    ipack = np.zeros((B, 128, IW), np.float32)
    locs_full = np.concatenate(
        [predicted_locs, np.zeros((B, npad, 4), np.float32)], axis=1)
    ipack[:, :, IO_LOC:IO_LOC + T * 4] = (
        locs_full.reshape(B, T, 128, 4).transpose(0, 2, 1, 3)
        .reshape(B, 128, T * 4))
    boxf = np.stack([bx1, by1, bx2, by2, barea], axis=1)    # [B, 5, K]
    ipack[:, :, IO_BB:IO_BB + 5 * K] = boxf.reshape(B, 1, 5 * K)
    lmask = (np.arange(C)[None, :, None] == labels_f[:, None, :])
    ipack[:, :C, IO_LM:IO_LM + K] = lmask.astype(np.float32)

    # const pack
    pad_pri = np.tile(np.array([-100.0, -100.0, 1.0, 1.0], np.float32),
                      (npad, 1))
    pri = np.concatenate([priors, pad_pri], axis=0)
    pcx, pcy, pw, ph = pri[:, 0], pri[:, 1], pri[:, 2], pri[:, 3]
    ptab = np.stack([
        pcx - pw / 2, pcy - ph / 2, pcx + pw / 2, pcy + ph / 2,
        pw * ph,
        pcx * (10.0 / pw), pcy * (10.0 / ph),
        10.0 / pw, 10.0 / ph,
        5.0 * np.log(pw), 5.0 * np.log(ph),
    ]).astype(np.float32)                                   # [11, PP]
    cst = np.zeros((128, CW), np.float32)
    # [11, PP] -> [11, T, 128] -> [128, 11, T]
    cst[:, CO_PT:CO_PT + 11 * T] = (
        ptab.reshape(11, T, 128).transpose(2, 0, 1).reshape(128, 11 * T))
    cst[:, CO_IDF:CO_IDF + 128] = np.eye(128, dtype=np.float32)
    cst[:, CO_IO15:CO_IO15 + 16] = np.concatenate(
        [np.arange(1, 16, dtype=np.float32) / 16.0, [999.0]])
    cst[:, CO_KV16:CO_KV16 + 16] = KV0 + KVS * np.arange(16, dtype=np.float32)
    cst[:, CO_THR:CO_THR + 16] = np.arange(16, dtype=np.float32)

    identb = _to_bf16(np.eye(128, dtype=np.float32))

    in_maps = []
    for c in range(NCORES):
        sl = slice(c * I, (c + 1) * I)
        in_maps.append({
            "sco": sco[sl],
            "ipack": np.ascontiguousarray(
                ipack[sl].transpose(1, 0, 2).reshape(128, I * IW)),
            "cst": cst,
            "identb": identb,
        })
    return in_maps


def combine_outputs(outs):
    """outs: list of 8 per-core [1,16] arrays -> scalar loss."""
    parts = np.concatenate([o.reshape(4, 4) for o in outs], axis=1)  # [4, 32]
    n_pos_total = parts[0].sum()
    box_sum = parts[1].sum()
    class_sum = parts[2].sum() + parts[3].sum()
    loss = class_sum / n_pos_total + box_sum / (n_pos_total * 4.0)
    return np.float32(loss)


_NC_CACHE = {}


def kernel(predicted_locs, predicted_scores, boxes, labels, priors_centers):
    if "nc" not in _NC_CACHE:
        _NC_CACHE["nc"] = build_nc()
    nc = _NC_CACHE["nc"]
    in_maps = prepare_inputs(predicted_locs, predicted_scores, boxes, labels,
                             priors_centers)
    res = run_bass_kernel_spmd(nc, in_maps, list(range(NCORES)))
    outs = [res.results[c]["out"] for c in range(NCORES)]
    return combine_outputs(outs)


if __name__ == "__main__":
    import reference as R

    inputs = {k: np.asarray(v) for k, v in R.setup_inputs().items()}
    print("loss =", kernel(**inputs))


# revision 20
# speedup vs baseline: 1.0302x; 1.0302x over previous
"""Trainium2 Bass kernel for SSD MultiBox loss (nn_ModelLoss_5970004541458).

Strategy: data-parallel over batch (32 images -> 8 cores x 4 images).
Per core, everything over the prior dim (P=8732, padded to 8960 = 70*128)
runs on-device:
  - jaccard matching in bf16 log-IoU space (monotone, so max/argmax/threshold
    comparisons are unchanged; threshold ln 0.5). Paired (x,y) ops halve the
    instruction count; bf16 doubles DVE throughput.
  - forced assignment via ADDITIVE sentinels ov + fmask*(100+4k): the 4-unit
    k spacing exceeds the ln-IoU range of forced points, so the largest k
    wins among colliding boxes (emulates the reference's last-wins scatter).
  - per-prior one-hot box gather via PE transpose + block-diag matmul (bf16)
  - CE: exp on ACT (bf16, 2 big chunks), class-sums on DVE (bf16),
    score-at-label via PE with sres stationary (81-col LDW, 16-col moving)
    and a CPU-precomputed label one-hot.
  - hard-negative mining via a 2-level 16-way counting grid with bounded-error
    boundary correction (no sort), per image.
All DRAM inputs are laid out per-partition-contiguous so every load is one
large DMA (128 descriptors of >=512B): one const pack, one image pack
(locs+boxes+labels-one-hot, CPU-pre-broadcast), one scores+qblk DMA per image.
Each core returns 16 partial sums; the host combines them into the loss.
"""
import sys

for _p in ("/opt/trn_rl_repo",):
    if _p not in sys.path:
        sys.path.insert(0, _p)

import numpy as np

import concourse.bass as bass
import concourse.tile as tile
from concourse import mybir
from concourse.bass_utils import run_bass_kernel_spmd

F32 = mybir.dt.float32
BF16 = mybir.dt.bfloat16
AX = mybir.AxisListType
OP = mybir.AluOpType
ACTF = mybir.ActivationFunctionType

B, P, C, K = 32, 8732, 81, 16
NCORES = 8
I = B // NCORES          # images per core = 4
PP = 8960                # padded priors = 70 * 128
T = PP // 128            # 70 prior tiles
T2 = 72                  # padded tile count for 128-col transpose blocks
NB = T2 * K // 128       # 9 transpose blocks of 128 (t,k)-columns
NCH = 2                  # score chunks per image (35 tiles each)
CT = T // NCH            # tiles per chunk = 35
THRESHOLD = 0.5
LN_THR = float(np.log(0.5))  # positives threshold in log-IoU space
KV0 = 100.0              # forced-assignment sentinel base (added to ln-IoU)
KVS = 4.0                # sentinel k spacing (> ln-IoU range of forced points)
NQ = 5                   # gathered quantities per box (cx, cy, 5lnw, 5lnh, pad)

# const pack column offsets (f32, [128, CW])
CO_PT = 0                # 11 prior-table rows x 70
CO_IDF = 770             # f32 identity 128
CO_IO15 = 898            # (1..15)/16 then 999
CO_KV16 = 914            # 100 + 4k
CO_THR = 930             # 0..15 level-1 mining thresholds
CW = 946

# image pack column offsets (f32, [128, I, IW])
IO_LOC = 0               # T2*4 locs (t-major, tail tiles zero)
IO_BB = 288              # 5x16 box rows (x1,y1,x2,y2,area), broadcast on CPU
IO_LM = 368              # label one-hot [81, 16] on partitions 0..80
IW = 384

# scores pack (bf16, [I, 128, SW])
SO_SC = 0                # 70*81 scores (t-major)
SO_QB = 5670             # 8*NQ block-diag gather stationary
SW = 5670 + 8 * NQ

_bf16 = np.dtype("uint16")  # bf16 carried as uint16 bit pattern if ml_dtypes absent
try:
    import ml_dtypes

    _bf16 = np.dtype(ml_dtypes.bfloat16)
except ImportError:
    ml_dtypes = None


def _to_bf16(x: np.ndarray) -> np.ndarray:
    if ml_dtypes is not None:
        return x.astype(ml_dtypes.bfloat16)
    u = x.astype(np.float32).view(np.uint32)
    rounded = ((u >> 16) + ((u >> 15) & 1)).astype(np.uint32)
    return (rounded & 0xFFFF).astype(np.uint16)


def _fixup_module(nc: bass.Bass) -> None:
    """Adapt the Tile-generated module to this container's walrus build.

    - EVENT_SEMAPHORE_RANGE_CLEAR is rejected ("ISA wrong length"); the
      preceding Drain(is_reset_sema) already resets the same range, so drop it.
    - Seq-only instructions accept fewer sync waits than Tile emits; hoist
      excess waits onto NoOps placed immediately before (same engine, so
      program order preserves semantics).
    """
    import bass_rust

    for f in nc.m.functions:
        for blk in f.blocks:
            newl = []
            for ins in blk.instructions:
                if getattr(ins, "op_name", None) == "EVENT_SEMAPHORE_RANGE_CLEAR":
                    continue
                si = ins.sync_info
                maxw = 1
                if si is not None and si.on_wait and len(si.on_wait) > maxw:
                    waits = list(si.on_wait)
                    extra, keep = waits[:-maxw], waits[-maxw:]
                    for j in range(0, len(extra), 1):
                        nop = mybir.InstNoOp(
                            name=f"{ins.name}-wsplit{j}", ins=[], outs=[],
                            engine=ins.engine)
                        nop.sync_info = bass_rust.SyncInfo(
                            on_wait=[extra[j]], on_update=[])
                        newl.append(nop)
                    ins.sync_info = bass_rust.SyncInfo(
                        on_wait=keep,
                        on_update=list(si.on_update) if si.on_update else [])
                newl.append(ins)
            blk.instructions = newl


def build_nc(fixup: bool = True) -> bass.Bass:
    nc = bass.Bass()

    d_sco = nc.dram_tensor("sco", [I, 128, SW], BF16, kind="ExternalInput")
    d_ipack = nc.dram_tensor("ipack", [128, I * IW], F32, kind="ExternalInput")
    d_cst = nc.dram_tensor("cst", [128, CW], F32, kind="ExternalInput")
    d_identb = nc.dram_tensor("identb", [128, 128], BF16, kind="ExternalInput")
    # out row layout (single partition): [np0..3, box0..3, cep0..3, mine0..3]
    d_out = nc.dram_tensor("out", [1, 16], F32, kind="ExternalOutput")

    from contextlib import ExitStack

    with tile.TileContext(nc) as tc, ExitStack() as es:
        cpool = es.enter_context(tc.tile_pool(name="consts", bufs=1))
        spool = es.enter_context(tc.tile_pool(name="scores", bufs=2))
        wpool = es.enter_context(tc.tile_pool(name="work", bufs=2))
        epool = es.enter_context(tc.tile_pool(name="exp", bufs=3))
        bpool = es.enter_context(tc.tile_pool(name="batched", bufs=1))
        pp_t = es.enter_context(tc.tile_pool(name="ps_t", bufs=1, space="PSUM"))
        pp_sel = es.enter_context(tc.tile_pool(name="ps_sel", bufs=1, space="PSUM"))
        pp_u = es.enter_context(tc.tile_pool(name="ps_u", bufs=1, space="PSUM"))
        pp_r = es.enter_context(tc.tile_pool(name="ps_r", bufs=1, space="PSUM"))

        # ---------------- constants (3 DMAs total) ----------------
        cpack = cpool.tile([128, CW], F32, tag="cpack")
        nc.sync.dma_start(out=cpack[:], in_=d_cst[:, :])
        ident = cpool.tile([128, 128], BF16, tag="ident")
        nc.sync.dma_start(out=ident[:], in_=d_identb[:, :])
        ipk = cpool.tile([128, I, IW], F32, tag="ipk")
        nc.sync.dma_start(out=ipk[:].rearrange("p i w -> p (i w)"),
                          in_=d_ipack[:, :])

        names = ["px1", "py1", "px2", "py2", "parea", "pcxn", "pcyn",
                 "ivx10", "ivy10", "lpw5", "lph5"]
        pt = {nm: cpack[:, CO_PT + r * T:CO_PT + (r + 1) * T]
              for r, nm in enumerate(names)}
        identf = cpack[:, CO_IDF:CO_IDF + 128]
        io15 = cpack[:, CO_IO15:CO_IO15 + 16]
        thrL1 = cpack[:, CO_THR:CO_THR + 16]

        ones_p = cpool.tile([128, 1], F32, tag="ones_p")
        nc.vector.memset(ones_p[:], 1.0)
        ones_r = cpool.tile([1, 128], F32, tag="ones_r")
        nc.vector.memset(ones_r[:], 1.0)
        ones_rb = cpool.tile([1, 128], BF16, tag="ones_rb")
        nc.vector.memset(ones_rb[:], 1.0)
        eps_b = cpool.tile([128, 1], F32, tag="eps_b")
        nc.vector.memset(eps_b[:], 1e-20)

        # bf16 staging copies of jaccard constants
        pt12b = cpool.tile([128, 2, T], BF16, tag="pt12b")
        nc.scalar.copy(pt12b[:].rearrange("p r t -> p (r t)"),
                       cpack[:, CO_PT:CO_PT + 2 * T])
        pt34b = cpool.tile([128, 2, T], BF16, tag="pt34b")
        nc.scalar.copy(pt34b[:].rearrange("p r t -> p (r t)"),
                       cpack[:, CO_PT + 2 * T:CO_PT + 4 * T])
        pareab = cpool.tile([128, T], BF16, tag="pareab")
        nc.scalar.copy(pareab[:], pt["parea"])
        kv1b = cpool.tile([128, 1], F32, tag="kv1b")
        nc.vector.memset(kv1b[:], KV0)

        def rowsum(dst_row_ap, src_ap, n):
            """[P, n] f32 -> [1, n] partition sum written to dst_row_ap."""
            ps = pp_r.tile([1, 128], F32, tag="red_row")
            nc.tensor.matmul(ps[:, :n], lhsT=ones_p[:src_ap.shape[0], :],
                             rhs=src_ap, start=True, stop=True)
            nc.scalar.copy(dst_row_ap, ps[:, :n])

        def bcast_row(dst_ap, row_ap, n, bf=False):
            """[1, n] -> [128, n] replicated."""
            ps = pp_r.tile([128, 128], F32, tag="red_bc")
            nc.tensor.matmul(ps[:, :n], lhsT=ones_rb[:] if bf else ones_r[:],
                             rhs=row_ap, start=True, stop=True)
            nc.scalar.copy(dst_ap, ps[:, :n])

        def maxreduce_row(dst_row_ap, src_ap, n):
            """[128, n] f32 -> [1, n] partition max written to dst_row_ap."""
            ps = pp_r.tile([128, 128], F32, tag="red_bc")
            nc.tensor.transpose(ps[:n, :], src_ap, identf)
            tsb = wpool.tile([128, 128], F32, tag="red_tsb")
            nc.scalar.copy(tsb[:n, :], ps[:n, :])
            mx = wpool.tile([128, 1], F32, tag="red_mx")
            nc.vector.tensor_reduce(out=mx[:n, :], in_=tsb[:n, :],
                                    axis=AX.X, op=OP.max)
            ps2 = pp_r.tile([1, 128], F32, tag="red_row")
            nc.tensor.transpose(ps2[:, :n], mx[:n, :], identf[:n, :n])
            nc.scalar.copy(dst_row_ap, ps2[:, :n])

        # IV4: [128, T2, 4] with d = (x, y, w, h); tail t>=T zeroed
        iv4 = cpool.tile([128, T2, 4], F32, tag="iv4")
        nc.vector.memset(iv4[:], 0.0)
        nc.vector.tensor_copy(iv4[:, :T, 0], pt["ivx10"])
        nc.vector.tensor_copy(iv4[:, :T, 1], pt["ivy10"])
        nc.vector.memset(iv4[:, :T, 2], 1.0)
        nc.vector.memset(iv4[:, :T, 3], 1.0)

        # persistent accumulators
        nprow = bpool.tile([1, I], F32, tag="nprow")
        scadd = bpool.tile([128, I, 4], F32, tag="scadd")   # fs, cn, lps, box
        scrow = bpool.tile([1, I, 4], F32, tag="scrow")
        bm4 = bpool.tile([128, I], F32, tag="bm4")
        bmrow = bpool.tile([1, I], F32, tag="bmrow")
        ufall = bpool.tile([C, I], F32, tag="ufall")
        uf4 = bpool.tile([1, I], F32, tag="uf4")
        out_sb = bpool.tile([1, 16], F32, tag="out_sb")

        for i in range(I):
            # ---------------- per-image load (1 DMA) ----------------
            sct = spool.tile([128, SW], BF16, tag="sct")
            nc.sync.dma_start(out=sct[:], in_=d_sco[i, :, :])
            sres = sct[:, :SO_QB].rearrange("p (t c) -> p t c", c=C)
            qblk = sct[:, SO_QB:SW]
            l4 = ipk[:, i, IO_LOC:IO_LOC + T2 * 4].rearrange(
                "p (t d) -> p t d", d=4)
            lmv = ipk[0:C, i, IO_LM:IO_LM + K]
            bbb = wpool.tile([128, 5, K], BF16, tag="bbb")
            nc.scalar.copy(bbb[:].rearrange("p a k -> p (a k)"),
                           ipk[:, i, IO_BB:IO_BB + 5 * K])

            # ---------------- jaccard, paired (x,y) in bf16 ----------------
            lt2 = wpool.tile([128, 2, T, K], BF16, tag="lt2")
            wh2 = wpool.tile([128, 2, T, K], BF16, tag="wh2")
            iu2 = wpool.tile([128, 2, T, K], BF16, tag="iu2")
            lnb = wpool.tile([128, 2, T, K], BF16, tag="lnb")
            ov = wpool.tile([128, T, K], BF16, tag="ov")
            nc.vector.tensor_tensor(
                out=lt2[:],
                in0=pt12b[:][:, :, :, None].broadcast_to([128, 2, T, K]),
                in1=bbb[:, 0:2, :][:, :, None, :].broadcast_to([128, 2, T, K]),
                op=OP.max)
            nc.vector.tensor_tensor(
                out=wh2[:],
                in0=pt34b[:][:, :, :, None].broadcast_to([128, 2, T, K]),
                in1=bbb[:, 2:4, :][:, :, None, :].broadcast_to([128, 2, T, K]),
                op=OP.min)
            nc.vector.tensor_sub(wh2[:], wh2[:], lt2[:])
            nc.scalar.activation(wh2[:], wh2[:], ACTF.Relu)
            nc.vector.tensor_mul(iu2[:, 0], wh2[:, 0], wh2[:, 1])
            nc.vector.tensor_tensor(
                out=iu2[:, 1],
                in0=pareab[:][:, :, None].broadcast_to([128, T, K]),
                in1=bbb[:, 4, :][:, None, :].broadcast_to([128, T, K]),
                op=OP.add)
            nc.vector.tensor_sub(iu2[:, 1], iu2[:, 1], iu2[:, 0])
            # log-space IoU: monotone, so comparisons unchanged
            nc.scalar.activation(lnb[:], iu2[:], ACTF.Ln, bias=eps_b[:])
            nc.vector.tensor_sub(ov[:], lnb[:, 0], lnb[:, 1])

            # ---------------- matching pass 2 ----------------
            m16 = wpool.tile([128, K], F32, tag="m16")
            m16r = wpool.tile([128, K], F32, tag="m16r")
            nc.vector.tensor_reduce(
                out=m16[:], in_=ov[:].rearrange("p t k -> p k t"),
                axis=AX.X, op=OP.max)
            m16row = wpool.tile([1, K], F32, tag="m16row")
            maxreduce_row(m16row[:], m16[:], K)
            bcast_row(m16r[:], m16row[:], K)
            fmask = wpool.tile([128, T, K], BF16, tag="fmask")
            nc.vector.tensor_tensor(
                out=fmask[:], in0=ov[:],
                in1=m16r[:][:, None, :].broadcast_to([128, T, K]),
                op=OP.is_equal)
            ovf = wpool.tile([128, T, K], BF16, tag="ovf")
            nc.vector.scalar_tensor_tensor(
                out=ovf[:], in0=fmask[:], scalar=kv1b[:], in1=ov[:],
                op0=OP.mult, op1=OP.add)
            pm = wpool.tile([128, T], BF16, tag="pm")
            nc.vector.tensor_reduce(out=pm[:], in_=ovf[:], axis=AX.X, op=OP.max)
            ohb = wpool.tile([128, T2 * K], BF16, tag="ohb")
            nc.vector.memset(ohb[:, T * K:], 0.0)
            nc.vector.tensor_tensor(
                out=ohb[:, :T * K].rearrange("p (t k) -> p t k", k=K),
                in0=ovf[:],
                in1=pm[:][:, :, None].broadcast_to([128, T, K]),
                op=OP.is_equal)
            pos = wpool.tile([128, T], F32, tag="pos")
            npt = wpool.tile([128, 1], F32, tag="npt")
            nc.vector.tensor_scalar(out=pos[:], in0=pm[:],
                                    scalar1=LN_THR, scalar2=None,
                                    op0=OP.is_ge, op1=OP.add,
                                    accum_out=npt[:])
            wmat = wpool.tile([128, T, K], BF16, tag="wmat")
            nc.vector.tensor_tensor(
                out=wmat[:],
                in0=ohb[:, :T * K].rearrange("p (t k) -> p t k", k=K),
                in1=pos[:][:, :, None].broadcast_to([128, T, K]),
                op=OP.mult)

            # n_pos for this image
            rowsum(nprow[:, i:i + 1], npt[:], 1)
            npb = wpool.tile([128, 1], F32, tag="npb")
            bcast_row(npb[:], nprow[:, i:i + 1], 1)
            k3b = wpool.tile([128, 1], F32, tag="k3b")
            nc.gpsimd.tensor_scalar(out=k3b[:], in0=npb[:], scalar1=3.0,
                                    scalar2=None, op0=OP.mult)

            # ---------------- box gather via PE ----------------
            ohT_ps = pp_t.tile([128, NB, 128], BF16, tag="ohT")
            for b in range(NB):
                nc.tensor.transpose(
                    ohT_ps[:, b, :],
                    ohb[:, b * 128:(b + 1) * 128],
                    ident[:])
            ohT_sb = wpool.tile([128, NB * 128], BF16, tag="ohT_sb")
            nc.scalar.copy(ohT_sb[:], ohT_ps[:].rearrange("p b n -> p (b n)"))

            sel_ps = pp_sel.tile([8 * NQ, NB, 128], F32, tag="sel")
            for b in range(NB):
                nc.tensor.matmul(sel_ps[:, b, :], lhsT=qblk[:],
                                 rhs=ohT_sb[:, b * 128:(b + 1) * 128],
                                 start=True, stop=True)
            sel_sb = wpool.tile([8 * NQ, NB * 128], BF16, tag="sel_sb")
            nc.scalar.copy(sel_sb[:], sel_ps[:].rearrange("p b n -> p (b n)"))
            bk_ps = pp_t.tile([128, NB, 8 * NQ], BF16, tag="ohT")
            for b in range(NB):
                nc.tensor.transpose(
                    bk_ps[:, b, :],
                    sel_sb[:, b * 128:(b + 1) * 128],
                    ident[:8 * NQ, :8 * NQ])
            selq = wpool.tile([128, NB * 8 * NQ], F32, tag="selq")
            nc.scalar.copy(selq[:], bk_ps[:].rearrange("p b n -> p (b n)"))
            # selq[p, (blk*40 + tb*5 + q)] = sel_q at t = blk*8+tb
            sel4 = selq[:].rearrange("p (t q) -> p t q", q=NQ)[:, :, 0:4]

            # ---------------- box L1 (l4 = locs + prior offsets, from CPU) ---
            tb1 = wpool.tile([128, T2, 4], F32, tag="tb1")
            nc.vector.tensor_mul(tb1[:], sel4, iv4[:])
            nc.vector.tensor_sub(tb1[:], l4, tb1[:])
            nc.vector.tensor_tensor(
                out=tb1[:, :T, :], in0=tb1[:, :T, :],
                in1=pos[:][:, :, None].broadcast_to([128, T, 4]),
                op=OP.mult)
            bacc = wpool.tile([128, 1], F32, tag="bacc")
            nc.scalar.activation(tb1[:], tb1[:], ACTF.Abs, accum_out=bacc[:])
            nc.scalar.copy(scadd[:, i, 3:4], bacc[:])

            # ------------- score at label: sres stationary on PE -------------
            u_ps = pp_u.tile([C, K], F32, tag="u")
            for t_ in range(T):
                nc.tensor.matmul(u_ps[:], lhsT=sres[:, t_, :],
                                 rhs=wmat[:, t_, :],
                                 start=(t_ == 0), stop=(t_ == T - 1))
            u_sb = wpool.tile([C, K], F32, tag="u_sb")
            nc.scalar.copy(u_sb[:], u_ps[:])
            ufx = wpool.tile([C, K], F32, tag="ufx")
            ufa = wpool.tile([C, 1], F32, tag="ufa")
            nc.vector.tensor_mul(ufx[:], u_sb[:], lmv)
            nc.vector.tensor_scalar(out=ufx[:], in0=ufx[:], scalar1=1.0,
                                    scalar2=None, op0=OP.mult, op1=OP.add,
                                    accum_out=ufa[:])
            nc.scalar.copy(ufall[:, i:i + 1], ufa[:])

            # ---------------- CE: exp on ACT + DVE reduces ----------------
            se = wpool.tile([128, T], BF16, tag="se")
            for ch in range(NCH):
                et = epool.tile([128, CT, C], BF16, tag="exps")
                nc.scalar.activation(
                    et[:], sres[:, ch * CT:(ch + 1) * CT, :], ACTF.Exp)
                with nc.allow_low_precision("bf16 lse; 2e-2 loss tolerance"):
                    nc.vector.tensor_reduce(
                        out=se[:, ch * CT:(ch + 1) * CT],
                        in_=et[:], axis=AX.X, op=OP.add)

            lse = wpool.tile([128, T], F32, tag="lse")
            nc.scalar.activation(lse[:], se[:], ACTF.Ln)
            ce0 = wpool.tile([128, T], F32, tag="ce0")
            nc.vector.tensor_sub(ce0[:], lse[:], sres[:, :, 0])
            cen = wpool.tile([128, T], F32, tag="cen")
            nc.vector.scalar_tensor_tensor(
                out=cen[:], in0=pos[:], scalar=THRESHOLD, in1=ce0[:],
                op0=OP.is_lt, op1=OP.mult)
            # ce_pos partial: sum(lse * pos) (minus U part in final combine)
            lpst = wpool.tile([128, T], F32, tag="lpst")
            lps = wpool.tile([128, 1], F32, tag="lps")
            nc.vector.scalar_tensor_tensor(
                out=lpst[:], in0=pos[:], scalar=1.0, in1=lse[:],
                op0=OP.mult, op1=OP.mult, accum_out=lps[:])
            nc.scalar.copy(scadd[:, i, 2:3], lps[:])

            # ---------------- mining (2-level 16-way grid) ----------------
            msk = wpool.tile([128, 16, T], F32, tag="msk")
            cnt16 = wpool.tile([128, 16], F32, tag="cnt16")
            nc.vector.tensor_tensor(
                out=msk[:],
                in0=cen[:][:, None, :].broadcast_to([128, 16, T]),
                in1=thrL1[:, :, None].broadcast_to([128, 16, T]),
                op=OP.is_gt)
            nc.vector.tensor_reduce(out=cnt16[:], in_=msk[:], axis=AX.X,
                                    op=OP.add)
            c1row = wpool.tile([1, 16], F32, tag="c1row")
            rowsum(c1row[:], cnt16[:], 16)
            cntr16 = wpool.tile([128, 16], F32, tag="cntr16")
            bcast_row(cntr16[:], c1row[:], 16)
            # lo = (#edges with count >= k) - 1   (edges j = 0..15)
            ge16 = wpool.tile([128, 16], F32, tag="ge16")
            lo1 = wpool.tile([128, 1], F32, tag="lo1")
            nc.vector.tensor_scalar(out=ge16[:], in0=cntr16[:],
                                    scalar1=k3b[:], scalar2=None,
                                    op0=OP.is_ge, op1=OP.add,
                                    accum_out=lo1[:])
            nc.vector.tensor_scalar(out=lo1[:], in0=lo1[:], scalar1=-1.0,
                                    scalar2=None, op0=OP.add)
            lop1 = wpool.tile([128, 1], F32, tag="lop1")
            nc.gpsimd.tensor_scalar(out=lop1[:], in0=lo1[:], scalar1=1.0 / 16,
                                    scalar2=None, op0=OP.add)
            # level 2: thresholds lo + m/16 (io15 has (1..15)/16 then +999)
            thr2 = wpool.tile([128, 16], F32, tag="thr2")
            nc.vector.tensor_scalar(out=thr2[:], in0=io15,
                                    scalar1=lo1[:], scalar2=None,
                                    op0=OP.add)
            msc2 = wpool.tile([128, 16, T], F32, tag="msc2")
            c2 = wpool.tile([128, 16], F32, tag="c2")
            nc.vector.tensor_tensor(
                out=msc2[:],
                in0=cen[:][:, None, :].broadcast_to([128, 16, T]),
                in1=thr2[:][:, :, None].broadcast_to([128, 16, T]),
                op=OP.is_gt)
            nc.vector.tensor_reduce(out=c2[:], in_=msc2[:], axis=AX.X,
                                    op=OP.add)
            c2row = wpool.tile([1, 16], F32, tag="c2row")
            rowsum(c2row[:], c2[:], 16)
            c2r = wpool.tile([128, 16], F32, tag="c2r")
            bcast_row(c2r[:], c2row[:], 16)
            ge2 = wpool.tile([128, 16], F32, tag="ge2")
            mc = wpool.tile([128, 1], F32, tag="mc")
            nc.vector.tensor_scalar(out=ge2[:], in0=c2r[:],
                                    scalar1=k3b[:], scalar2=None,
                                    op0=OP.is_ge, op1=OP.add, accum_out=mc[:])
            hi1 = wpool.tile([128, 1], F32, tag="hi1")
            nc.vector.tensor_scalar(out=hi1[:], in0=mc[:],
                                    scalar1=1.0 / 16, scalar2=lop1[:],
                                    op0=OP.mult, op1=OP.add)
            # F(hi), count(hi), boundary max
            fsc = wpool.tile([128, T], F32, tag="fsc")
            fsa = wpool.tile([128, 1], F32, tag="fsa")
            nc.vector.scalar_tensor_tensor(
                out=fsc[:], in0=cen[:], scalar=hi1[:],
                in1=cen[:], op0=OP.is_gt, op1=OP.mult,
                accum_out=fsa[:])
            nc.scalar.copy(scadd[:, i, 0:1], fsa[:])
            cna = wpool.tile([128, 1], F32, tag="cna")
            nc.vector.tensor_scalar(out=fsc[:], in0=cen[:],
                                    scalar1=hi1[:], scalar2=None,
                                    op0=OP.is_gt, op1=OP.add, accum_out=cna[:])
            nc.scalar.copy(scadd[:, i, 1:2], cna[:])
            nc.vector.scalar_tensor_tensor(
                out=fsc[:], in0=cen[:], scalar=hi1[:],
                in1=cen[:], op0=OP.is_le, op1=OP.mult)
            bmt = wpool.tile([128, 1], F32, tag="bmt")
            nc.vector.tensor_reduce(out=bmt[:], in_=fsc[:], axis=AX.X, op=OP.max)
            nc.scalar.copy(bm4[:, i:i + 1], bmt[:])

        # ---------------- final combine (partition 0) ----------------
        rowsum(scrow[:].rearrange("p i s -> p (i s)"),
               scadd[:].rearrange("p i s -> p (i s)"), I * 4)
        maxreduce_row(bmrow[:], bm4[:], I)
        rowsum(uf4[:], ufall[:], I)

        k34r = bpool.tile([1, I], F32, tag="k34r")
        nc.vector.tensor_scalar(out=k34r[:], in0=nprow[:], scalar1=3.0,
                                scalar2=None, op0=OP.mult)
        r4 = bpool.tile([1, I], F32, tag="r4")
        nc.vector.tensor_sub(r4[:], k34r[:], scrow[:, :, 1])
        nc.vector.tensor_mul(r4[:], r4[:], bmrow[:])
        nc.vector.tensor_add(r4[:], r4[:], scrow[:, :, 0])   # mine sums
        cep = bpool.tile([1, I], F32, tag="cep")
        nc.vector.tensor_sub(cep[:], scrow[:, :, 2], uf4[:])  # ce_pos sums
        nc.vector.tensor_copy(out_sb[:, 0:4], nprow[:])
        nc.vector.tensor_copy(out_sb[:, 4:8], scrow[:, :, 3])
        nc.vector.tensor_copy(out_sb[:, 8:12], cep[:])
        nc.vector.tensor_copy(out_sb[:, 12:16], r4[:])
        nc.sync.dma_start(out=d_out[:, :], in_=out_sb[:])

    if fixup:
        _fixup_module(nc)
    return nc


def prepare_inputs(predicted_locs, predicted_scores, boxes, labels,
                   priors_centers):
    """Shard + marshal the full inputs into 8 per-core in_maps.

    All DRAM layouts are per-partition contiguous (partition-major), so
    every SBUF partition reads one contiguous chunk per DMA.
    """
    predicted_locs = np.asarray(predicted_locs, np.float32)
    predicted_scores = np.asarray(predicted_scores, np.float32)
    boxes = np.asarray(boxes, np.float32)
    labels_f = np.asarray(labels).astype(np.int64)
    priors = np.asarray(priors_centers, np.float32)

    npad = PP - P
    # scores: pad rows have class0=0, others -50 -> lse=0, S0=0, ce0=0 exactly
    pad_scores = np.full((B, npad, C), -50.0, np.float32)
    pad_scores[:, :, 0] = 0.0
    scores_p = np.concatenate([predicted_scores, pad_scores], axis=1)
    # [B, PP, C] -> [B, T, 128, C] -> [B, 128, T, C] -> [B, 128, T*C]
    scores_pm = scores_p.reshape(B, T, 128, C).transpose(0, 2, 1, 3)

    bx1, by1, bx2, by2 = (boxes[:, :, d] for d in range(4))
    barea = (bx2 - bx1) * (by2 - by1)
    q5 = np.stack([
        (bx1 + bx2) / 2, (by1 + by2) / 2,
        5.0 * np.log(bx2 - bx1), 5.0 * np.log(by2 - by1),
        np.zeros_like(bx1),
    ], axis=2).astype(np.float32)                           # [B, K, 5]
    qblk = np.zeros((B, 128, 8 * NQ), np.float32)
    for tb in range(8):
        qblk[:, tb * K:(tb + 1) * K, tb * NQ:(tb + 1) * NQ] = q5

    sco = np.zeros((B, 128, SW), np.float32)
    sco[:, :, :SO_QB] = scores_pm.reshape(B, 128, T * C)
    sco[:, :, SO_QB:] = qblk
    sco = _to_bf16(sco)

    # image pack: locs (t-major, tail zero) + broadcast box rows + label 1-hot
    ipack = np.zeros((B, 128, IW), np.float32)
    pcx0, pcy0, pw0, ph0 = (np.asarray(priors_centers, np.float32)[:, d]
                            for d in range(4))
    pofs = np.stack([pcx0 * (10.0 / pw0), pcy0 * (10.0 / ph0),
                     5.0 * np.log(pw0), 5.0 * np.log(ph0)], axis=1)  # [P, 4]
    locs_full = np.concatenate(
        [predicted_locs + pofs[None, :, :],
         np.zeros((B, npad, 4), np.float32)], axis=1)
    ipack[:, :, IO_LOC:IO_LOC + T * 4] = (
        locs_full.reshape(B, T, 128, 4).transpose(0, 2, 1, 3)
        .reshape(B, 128, T * 4))
    boxf = np.stack([bx1, by1, bx2, by2, barea], axis=1)    # [B, 5, K]
    ipack[:, :, IO_BB:IO_BB + 5 * K] = boxf.reshape(B, 1, 5 * K)
    lmask = (np.arange(C)[None, :, None] == labels_f[:, None, :])
    ipack[:, :C, IO_LM:IO_LM + K] = lmask.astype(np.float32)

    # const pack
    pad_pri = np.tile(np.array([-100.0, -100.0, 1.0, 1.0], np.float32),
                      (npad, 1))
    pri = np.concatenate([priors, pad_pri], axis=0)
    pcx, pcy, pw, ph = pri[:, 0], pri[:, 1], pri[:, 2], pri[:, 3]
    ptab = np.stack([
        pcx - pw / 2, pcy - ph / 2, pcx + pw / 2, pcy + ph / 2,
        pw * ph,
        pcx * (10.0 / pw), pcy * (10.0 / ph),
        10.0 / pw, 10.0 / ph,
        5.0 * np.log(pw), 5.0 * np.log(ph),
    ]).astype(np.float32)                                   # [11, PP]
    cst = np.zeros((128, CW), np.float32)
    # [11, PP] -> [11, T, 128] -> [128, 11, T]
    cst[:, CO_PT:CO_PT + 11 * T] = (
        ptab.reshape(11, T, 128).transpose(2, 0, 1).reshape(128, 11 * T))
    cst[:, CO_IDF:CO_IDF + 128] = np.eye(128, dtype=np.float32)
    cst[:, CO_IO15:CO_IO15 + 16] = np.concatenate(
        [np.arange(1, 16, dtype=np.float32) / 16.0, [999.0]])
    cst[:, CO_KV16:CO_KV16 + 16] = KV0 + KVS * np.arange(16, dtype=np.float32)
    cst[:, CO_THR:CO_THR + 16] = np.arange(16, dtype=np.float32)

    identb = _to_bf16(np.eye(128, dtype=np.float32))

    in_maps = []
    for c in range(NCORES):
        sl = slice(c * I, (c + 1) * I)
        in_maps.append({
            "sco": sco[sl],
            "ipack": np.ascontiguousarray(
                ipack[sl].transpose(1, 0, 2).reshape(128, I * IW)),
            "cst": cst,
            "identb": identb,
        })
    return in_maps


def combine_outputs(outs):
    """outs: list of 8 per-core [1,16] arrays -> scalar loss."""
    parts = np.concatenate([o.reshape(4, 4) for o in outs], axis=1)  # [4, 32]
    n_pos_total = parts[0].sum()
    box_sum = parts[1].sum()
    class_sum = parts[2].sum() + parts[3].sum()
    loss = class_sum / n_pos_total + box_sum / (n_pos_total * 4.0)
    return np.float32(loss)


_NC_CACHE = {}


def kernel(predicted_locs, predicted_scores, boxes, labels, priors_centers):
    if "nc" not in _NC_CACHE:
        _NC_CACHE["nc"] = build_nc()
    nc = _NC_CACHE["nc"]
    in_maps = prepare_inputs(predicted_locs, predicted_scores, boxes, labels,
                             priors_centers)
    res = run_bass_kernel_spmd(nc, in_maps, list(range(NCORES)))
    outs = [res.results[c]["out"] for c in range(NCORES)]
    return combine_outputs(outs)


if __name__ == "__main__":
    import reference as R

    inputs = {k: np.asarray(v) for k, v in R.setup_inputs().items()}
    print("loss =", kernel(**inputs))


# revision 22
# speedup vs baseline: 1.0556x; 1.0246x over previous
"""Trainium2 Bass kernel for SSD MultiBox loss (nn_ModelLoss_5970004541458).

Strategy: data-parallel over batch (32 images -> 8 cores x 4 images).
Per core, everything over the prior dim (P=8732, padded to 8960 = 70*128)
runs on-device:
  - jaccard matching in bf16 log-IoU space (monotone, so max/argmax/threshold
    comparisons are unchanged; threshold ln 0.5). Paired (x,y) ops halve the
    instruction count; bf16 doubles DVE throughput.
  - forced assignment via ADDITIVE sentinels ov + fmask*(100+4k): the 4-unit
    k spacing exceeds the ln-IoU range of forced points, so the largest k
    wins among colliding boxes (emulates the reference's last-wins scatter).
  - per-prior one-hot box gather via PE transpose + block-diag matmul (bf16)
  - CE: exp on ACT (bf16, 2 big chunks), class-sums on DVE (bf16),
    score-at-label via PE with sres stationary (81-col LDW, 16-col moving)
    and a CPU-precomputed label one-hot.
  - hard-negative mining via a 2-level 16-way counting grid with bounded-error
    boundary correction (no sort), per image.
All DRAM inputs are laid out per-partition-contiguous so every load is one
large DMA (128 descriptors of >=512B): one const pack, one image pack
(locs+boxes+labels-one-hot, CPU-pre-broadcast), one scores+qblk DMA per image.
Each core returns 16 partial sums; the host combines them into the loss.
"""
import sys

for _p in ("/opt/trn_rl_repo",):
    if _p not in sys.path:
        sys.path.insert(0, _p)

import numpy as np

import concourse.bass as bass
import concourse.tile as tile
from concourse import mybir
from concourse.bass_utils import run_bass_kernel_spmd

F32 = mybir.dt.float32
BF16 = mybir.dt.bfloat16
AX = mybir.AxisListType
OP = mybir.AluOpType
ACTF = mybir.ActivationFunctionType

B, P, C, K = 32, 8732, 81, 16
NCORES = 8
I = B // NCORES          # images per core = 4
PP = 8960                # padded priors = 70 * 128
T = PP // 128            # 70 prior tiles
T2 = 72                  # padded tile count for 128-col transpose blocks
NB = T2 * K // 128       # 9 transpose blocks of 128 (t,k)-columns
NCH = 2                  # score chunks per image (35 tiles each)
CT = T // NCH            # tiles per chunk = 35
THRESHOLD = 0.5
LN_THR = float(np.log(0.5))  # positives threshold in log-IoU space
KV0 = 100.0              # forced-assignment sentinel base (added to ln-IoU)
KVS = 4.0                # sentinel k spacing (> ln-IoU range of forced points)
NQ = 5                   # gathered quantities per box (cx, cy, 5lnw, 5lnh, pad)

# const pack column offsets (f32, [128, CW])
CO_PT = 0                # 11 prior-table rows x 70
CO_IDF = 770             # f32 identity 128
CO_IO15 = 898            # (1..15)/16 then 999
CO_KV16 = 914            # 100 + 4k
CO_THR = 930             # 0..15 level-1 mining thresholds
CW = 946

# image pack column offsets (f32, [128, I, IW])
IO_LOC = 0               # T2*4 locs (t-major, tail tiles zero)
IO_BB = 288              # 5x16 box rows (x1,y1,x2,y2,area), broadcast on CPU
IO_LM = 368              # label one-hot [81, 16] on partitions 0..80
IW = 384

# scores pack (bf16, [I, 128, SW])
SO_SC = 0                # 70*81 scores (t-major)
SO_QB = 5670             # 8*NQ block-diag gather stationary
SW = 5670 + 8 * NQ

_bf16 = np.dtype("uint16")  # bf16 carried as uint16 bit pattern if ml_dtypes absent
try:
    import ml_dtypes

    _bf16 = np.dtype(ml_dtypes.bfloat16)
except ImportError:
    ml_dtypes = None


def _to_bf16(x: np.ndarray) -> np.ndarray:
    if ml_dtypes is not None:
        return x.astype(ml_dtypes.bfloat16)
    u = x.astype(np.float32).view(np.uint32)
    rounded = ((u >> 16) + ((u >> 15) & 1)).astype(np.uint32)
    return (rounded & 0xFFFF).astype(np.uint16)


def _fixup_module(nc: bass.Bass) -> None:
    """Adapt the Tile-generated module to this container's walrus build.

    - EVENT_SEMAPHORE_RANGE_CLEAR is rejected ("ISA wrong length"); the
      preceding Drain(is_reset_sema) already resets the same range, so drop it.
    - Seq-only instructions accept fewer sync waits than Tile emits; hoist
      excess waits onto NoOps placed immediately before (same engine, so
      program order preserves semantics).
    """
    import bass_rust

    for f in nc.m.functions:
        for blk in f.blocks:
            newl = []
            for ins in blk.instructions:
                if getattr(ins, "op_name", None) == "EVENT_SEMAPHORE_RANGE_CLEAR":
                    continue
                si = ins.sync_info
                maxw = 1
                if si is not None and si.on_wait and len(si.on_wait) > maxw:
                    waits = list(si.on_wait)
                    extra, keep = waits[:-maxw], waits[-maxw:]
                    for j in range(0, len(extra), 1):
                        nop = mybir.InstNoOp(
                            name=f"{ins.name}-wsplit{j}", ins=[], outs=[],
                            engine=ins.engine)
                        nop.sync_info = bass_rust.SyncInfo(
                            on_wait=[extra[j]], on_update=[])
                        newl.append(nop)
                    ins.sync_info = bass_rust.SyncInfo(
                        on_wait=keep,
                        on_update=list(si.on_update) if si.on_update else [])
                newl.append(ins)
            blk.instructions = newl


def build_nc(fixup: bool = True) -> bass.Bass:
    nc = bass.Bass()

    d_sco = nc.dram_tensor("sco", [I, 128, SW], BF16, kind="ExternalInput")
    d_ipack = nc.dram_tensor("ipack", [128, I * IW], F32, kind="ExternalInput")
    d_cst = nc.dram_tensor("cst", [128, CW], F32, kind="ExternalInput")
    d_identb = nc.dram_tensor("identb", [128, 128], BF16, kind="ExternalInput")
    # out row layout (single partition): [np0..3, box0..3, cep0..3, mine0..3]
    d_out = nc.dram_tensor("out", [1, 16], F32, kind="ExternalOutput")

    from contextlib import ExitStack

    with tile.TileContext(nc) as tc, ExitStack() as es:
        cpool = es.enter_context(tc.tile_pool(name="consts", bufs=1))
        spool = es.enter_context(tc.tile_pool(name="scores", bufs=2))
        wpool = es.enter_context(tc.tile_pool(name="work", bufs=3))
        epool = es.enter_context(tc.tile_pool(name="exp", bufs=3))
        bpool = es.enter_context(tc.tile_pool(name="batched", bufs=1))
        pp_t = es.enter_context(tc.tile_pool(name="ps_t", bufs=1, space="PSUM"))
        pp_sel = es.enter_context(tc.tile_pool(name="ps_sel", bufs=1, space="PSUM"))
        pp_u = es.enter_context(tc.tile_pool(name="ps_u", bufs=1, space="PSUM"))
        pp_r = es.enter_context(tc.tile_pool(name="ps_r", bufs=2, space="PSUM"))

        # ---------------- constants (3 DMAs total) ----------------
        cpack = cpool.tile([128, CW], F32, tag="cpack")
        nc.sync.dma_start(out=cpack[:], in_=d_cst[:, :])
        ident = cpool.tile([128, 128], BF16, tag="ident")
        nc.sync.dma_start(out=ident[:], in_=d_identb[:, :])
        ipk = cpool.tile([128, I, IW], F32, tag="ipk")
        nc.sync.dma_start(out=ipk[:].rearrange("p i w -> p (i w)"),
                          in_=d_ipack[:, :])

        names = ["px1", "py1", "px2", "py2", "parea", "pcxn", "pcyn",
                 "ivx10", "ivy10", "lpw5", "lph5"]
        pt = {nm: cpack[:, CO_PT + r * T:CO_PT + (r + 1) * T]
              for r, nm in enumerate(names)}
        identf = cpack[:, CO_IDF:CO_IDF + 128]
        io15 = cpack[:, CO_IO15:CO_IO15 + 16]
        thrL1 = cpack[:, CO_THR:CO_THR + 16]

        ones_p = cpool.tile([128, 1], F32, tag="ones_p")
        nc.vector.memset(ones_p[:], 1.0)
        ones_r = cpool.tile([1, 128], F32, tag="ones_r")
        nc.vector.memset(ones_r[:], 1.0)
        ones_rb = cpool.tile([1, 128], BF16, tag="ones_rb")
        nc.vector.memset(ones_rb[:], 1.0)
        eps_b = cpool.tile([128, 1], F32, tag="eps_b")
        nc.vector.memset(eps_b[:], 1e-20)

        # bf16 staging copies of jaccard constants
        pt12b = cpool.tile([128, 2, T], BF16, tag="pt12b")
        nc.scalar.copy(pt12b[:].rearrange("p r t -> p (r t)"),
                       cpack[:, CO_PT:CO_PT + 2 * T])
        pt34b = cpool.tile([128, 2, T], BF16, tag="pt34b")
        nc.scalar.copy(pt34b[:].rearrange("p r t -> p (r t)"),
                       cpack[:, CO_PT + 2 * T:CO_PT + 4 * T])
        pareab = cpool.tile([128, T], BF16, tag="pareab")
        nc.scalar.copy(pareab[:], pt["parea"])
        kv1b = cpool.tile([128, 1], F32, tag="kv1b")
        nc.vector.memset(kv1b[:], KV0)

        def rowsum(dst_row_ap, src_ap, n):
            """[P, n] f32 -> [1, n] partition sum written to dst_row_ap."""
            ps = pp_r.tile([128, 128], F32, tag="red_bc")
            nc.tensor.matmul(ps[0:1, :n], lhsT=ones_p[:src_ap.shape[0], :],
                             rhs=src_ap, start=True, stop=True)
            nc.scalar.copy(dst_row_ap, ps[0:1, :n])

        def bcast_row(dst_ap, row_ap, n, bf=False):
            """[1, n] -> [128, n] replicated."""
            ps = pp_r.tile([128, 128], F32, tag="red_bc")
            nc.tensor.matmul(ps[:, :n], lhsT=ones_rb[:] if bf else ones_r[:],
                             rhs=row_ap, start=True, stop=True)
            nc.scalar.copy(dst_ap, ps[:, :n])

        def maxreduce_row(dst_row_ap, src_ap, n):
            """[128, n] f32 -> [1, n] partition max written to dst_row_ap."""
            ps = pp_r.tile([128, 128], F32, tag="red_bc")
            nc.tensor.transpose(ps[:n, :], src_ap, identf)
            tsb = wpool.tile([128, 128], F32, tag="red_tsb")
            nc.scalar.copy(tsb[:n, :], ps[:n, :])
            mx = wpool.tile([128, 1], F32, tag="red_mx")
            nc.vector.tensor_reduce(out=mx[:n, :], in_=tsb[:n, :],
                                    axis=AX.X, op=OP.max)
            ps2 = pp_r.tile([128, 128], F32, tag="red_bc")
            nc.tensor.transpose(ps2[0:1, :n], mx[:n, :], identf[:n, :n])
            nc.scalar.copy(dst_row_ap, ps2[0:1, :n])

        # IV4: [128, T2, 4] with d = (x, y, w, h); tail t>=T zeroed
        iv4 = cpool.tile([128, T2, 4], F32, tag="iv4")
        nc.vector.memset(iv4[:], 0.0)
        nc.vector.tensor_copy(iv4[:, :T, 0], pt["ivx10"])
        nc.vector.tensor_copy(iv4[:, :T, 1], pt["ivy10"])
        nc.vector.memset(iv4[:, :T, 2], 1.0)
        nc.vector.memset(iv4[:, :T, 3], 1.0)

        # persistent accumulators
        nprow = bpool.tile([1, I], F32, tag="nprow")
        scadd = bpool.tile([128, I, 4], F32, tag="scadd")   # fs, cn, lps, box
        scrow = bpool.tile([1, I, 4], F32, tag="scrow")
        bm4 = bpool.tile([128, I], F32, tag="bm4")
        bmrow = bpool.tile([1, I], F32, tag="bmrow")
        ufall = bpool.tile([C, I], F32, tag="ufall")
        uf4 = bpool.tile([1, I], F32, tag="uf4")
        out_sb = bpool.tile([1, 16], F32, tag="out_sb")

        for i in range(I):
            # ---------------- per-image load (1 DMA) ----------------
            sct = spool.tile([128, SW], BF16, tag="sct")
            nc.sync.dma_start(out=sct[:], in_=d_sco[i, :, :])
            sres = sct[:, :SO_QB].rearrange("p (t c) -> p t c", c=C)
            qblk = sct[:, SO_QB:SW]
            l4 = ipk[:, i, IO_LOC:IO_LOC + T2 * 4].rearrange(
                "p (t d) -> p t d", d=4)
            lmv = ipk[0:C, i, IO_LM:IO_LM + K]
            bbb = wpool.tile([128, 5, K], BF16, tag="bbb")
            nc.scalar.copy(bbb[:].rearrange("p a k -> p (a k)"),
                           ipk[:, i, IO_BB:IO_BB + 5 * K])

            # ---------------- jaccard, paired (x,y) in bf16 ----------------
            wh2 = wpool.tile([128, 2, T, K], BF16, tag="wh2")
            iu2 = wpool.tile([128, 2, T, K], BF16, tag="iu2")
            lnb = wpool.tile([128, 2, T, K], BF16, tag="lnb")
            ov = wpool.tile([128, T, K], BF16, tag="ov")
            nc.vector.tensor_tensor(
                out=iu2[:],
                in0=pt12b[:][:, :, :, None].broadcast_to([128, 2, T, K]),
                in1=bbb[:, 0:2, :][:, :, None, :].broadcast_to([128, 2, T, K]),
                op=OP.max)
            nc.vector.tensor_tensor(
                out=wh2[:],
                in0=pt34b[:][:, :, :, None].broadcast_to([128, 2, T, K]),
                in1=bbb[:, 2:4, :][:, :, None, :].broadcast_to([128, 2, T, K]),
                op=OP.min)
            nc.vector.tensor_sub(wh2[:], wh2[:], iu2[:])
            nc.scalar.activation(wh2[:], wh2[:], ACTF.Relu)
            nc.vector.tensor_mul(iu2[:, 0], wh2[:, 0], wh2[:, 1])
            nc.vector.tensor_tensor(
                out=iu2[:, 1],
                in0=pareab[:][:, :, None].broadcast_to([128, T, K]),
                in1=bbb[:, 4, :][:, None, :].broadcast_to([128, T, K]),
                op=OP.add)
            nc.vector.tensor_sub(iu2[:, 1], iu2[:, 1], iu2[:, 0])
            # log-space IoU: monotone, so comparisons unchanged
            nc.scalar.activation(lnb[:], iu2[:], ACTF.Ln, bias=eps_b[:])
            nc.vector.tensor_sub(ov[:], lnb[:, 0], lnb[:, 1])

            # ---------------- matching pass 2 ----------------
            m16 = wpool.tile([128, K], F32, tag="m16")
            m16r = wpool.tile([128, K], F32, tag="m16r")
            nc.vector.tensor_reduce(
                out=m16[:], in_=ov[:].rearrange("p t k -> p k t"),
                axis=AX.X, op=OP.max)
            m16row = wpool.tile([1, K], F32, tag="m16row")
            maxreduce_row(m16row[:], m16[:], K)
            bcast_row(m16r[:], m16row[:], K)
            fmask = wpool.tile([128, T, K], BF16, tag="fmask")
            nc.vector.tensor_tensor(
                out=fmask[:], in0=ov[:],
                in1=m16r[:][:, None, :].broadcast_to([128, T, K]),
                op=OP.is_equal)
            ovf = wpool.tile([128, T, K], BF16, tag="ovf")
            nc.vector.scalar_tensor_tensor(
                out=ovf[:], in0=fmask[:], scalar=kv1b[:], in1=ov[:],
                op0=OP.mult, op1=OP.add)
            pm = wpool.tile([128, T], BF16, tag="pm")
            nc.vector.tensor_reduce(out=pm[:], in_=ovf[:], axis=AX.X, op=OP.max)
            ohb = wpool.tile([128, T2 * K], BF16, tag="ohb")
            nc.vector.memset(ohb[:, T * K:], 0.0)
            nc.vector.tensor_tensor(
                out=ohb[:, :T * K].rearrange("p (t k) -> p t k", k=K),
                in0=ovf[:],
                in1=pm[:][:, :, None].broadcast_to([128, T, K]),
                op=OP.is_equal)
            pos = wpool.tile([128, T], F32, tag="pos")
            npt = wpool.tile([128, 1], F32, tag="npt")
            nc.vector.tensor_scalar(out=pos[:], in0=pm[:],
                                    scalar1=LN_THR, scalar2=None,
                                    op0=OP.is_ge, op1=OP.add,
                                    accum_out=npt[:])
            wmat = wpool.tile([128, T, K], BF16, tag="wmat")
            nc.vector.tensor_tensor(
                out=wmat[:],
                in0=ohb[:, :T * K].rearrange("p (t k) -> p t k", k=K),
                in1=pos[:][:, :, None].broadcast_to([128, T, K]),
                op=OP.mult)

            # n_pos for this image
            rowsum(nprow[:, i:i + 1], npt[:], 1)
            npb = wpool.tile([128, 1], F32, tag="npb")
            bcast_row(npb[:], nprow[:, i:i + 1], 1)
            k3b = wpool.tile([128, 1], F32, tag="k3b")
            nc.gpsimd.tensor_scalar(out=k3b[:], in0=npb[:], scalar1=3.0,
                                    scalar2=None, op0=OP.mult)

            # ---------------- box gather via PE ----------------
            ohT_ps = pp_t.tile([128, NB, 128], BF16, tag="ohT")
            for b in range(NB):
                nc.tensor.transpose(
                    ohT_ps[:, b, :],
                    ohb[:, b * 128:(b + 1) * 128],
                    ident[:])
            ohT_sb = wpool.tile([128, NB * 128], BF16, tag="ohT_sb")
            nc.scalar.copy(ohT_sb[:], ohT_ps[:].rearrange("p b n -> p (b n)"))

            sel_ps = pp_sel.tile([8 * NQ, NB, 128], F32, tag="sel")
            for b in range(NB):
                nc.tensor.matmul(sel_ps[:, b, :], lhsT=qblk[:],
                                 rhs=ohT_sb[:, b * 128:(b + 1) * 128],
                                 start=True, stop=True)
            sel_sb = wpool.tile([8 * NQ, NB * 128], BF16, tag="sel_sb")
            nc.scalar.copy(sel_sb[:], sel_ps[:].rearrange("p b n -> p (b n)"))
            bk_ps = pp_t.tile([128, NB, 8 * NQ], BF16, tag="ohT")
            for b in range(NB):
                nc.tensor.transpose(
                    bk_ps[:, b, :],
                    sel_sb[:, b * 128:(b + 1) * 128],
                    ident[:8 * NQ, :8 * NQ])
            # bk_ps[p, (blk*40 + tb*5 + q)] = sel_q at t = blk*8+tb; read PSUM
            sel4 = (bk_ps[:].rearrange("p b n -> p (b n)")
                    .rearrange("p (t q) -> p t q", q=NQ)[:, :, 0:4])

            # ---------------- box L1 (l4 = locs + prior offsets, from CPU) ---
            tb1 = wpool.tile([128, T2, 4], F32, tag="tb1")
            nc.vector.tensor_mul(tb1[:], sel4, iv4[:])
            nc.vector.tensor_sub(tb1[:], l4, tb1[:])
            nc.vector.tensor_tensor(
                out=tb1[:, :T, :], in0=tb1[:, :T, :],
                in1=pos[:][:, :, None].broadcast_to([128, T, 4]),
                op=OP.mult)
            bacc = wpool.tile([128, 1], F32, tag="bacc")
            nc.scalar.activation(tb1[:], tb1[:], ACTF.Abs, accum_out=bacc[:])
            nc.scalar.copy(scadd[:, i, 3:4], bacc[:])

            # ------------- score at label: sres stationary on PE -------------
            u_ps = pp_u.tile([C, K], F32, tag="u")
            for t_ in range(T):
                nc.tensor.matmul(u_ps[:], lhsT=sres[:, t_, :],
                                 rhs=wmat[:, t_, :],
                                 start=(t_ == 0), stop=(t_ == T - 1))
            ufx = wpool.tile([C, K], F32, tag="ufx")
            ufa = wpool.tile([C, 1], F32, tag="ufa")
            nc.vector.tensor_mul(ufx[:], u_ps[:], lmv)
            nc.vector.tensor_scalar(out=ufx[:], in0=ufx[:], scalar1=1.0,
                                    scalar2=None, op0=OP.mult, op1=OP.add,
                                    accum_out=ufa[:])
            nc.scalar.copy(ufall[:, i:i + 1], ufa[:])

            # ---------------- CE: exp on ACT + DVE reduces ----------------
            se = wpool.tile([128, T], BF16, tag="se")
            for ch in range(NCH):
                et = epool.tile([128, CT, C], BF16, tag="exps")
                nc.scalar.activation(
                    et[:], sres[:, ch * CT:(ch + 1) * CT, :], ACTF.Exp)
                with nc.allow_low_precision("bf16 lse; 2e-2 loss tolerance"):
                    nc.vector.tensor_reduce(
                        out=se[:, ch * CT:(ch + 1) * CT],
                        in_=et[:], axis=AX.X, op=OP.add)

            lse = wpool.tile([128, T], F32, tag="lse")
            nc.scalar.activation(lse[:], se[:], ACTF.Ln)
            ce0 = wpool.tile([128, T], F32, tag="ce0")
            nc.vector.tensor_sub(ce0[:], lse[:], sres[:, :, 0])
            cen = wpool.tile([128, T], F32, tag="cen")
            nc.vector.scalar_tensor_tensor(
                out=cen[:], in0=pos[:], scalar=THRESHOLD, in1=ce0[:],
                op0=OP.is_lt, op1=OP.mult)
            # ce_pos partial: sum(lse * pos) (minus U part in final combine)
            lpst = wpool.tile([128, T], F32, tag="lpst")
            lps = wpool.tile([128, 1], F32, tag="lps")
            nc.vector.scalar_tensor_tensor(
                out=lpst[:], in0=pos[:], scalar=1.0, in1=lse[:],
                op0=OP.mult, op1=OP.mult, accum_out=lps[:])
            nc.scalar.copy(scadd[:, i, 2:3], lps[:])

            # ---------------- mining (2-level 16-way grid) ----------------
            msk = wpool.tile([128, 16, T], F32, tag="msk")
            cnt16 = wpool.tile([128, 16], F32, tag="cnt16")
            nc.vector.tensor_tensor(
                out=msk[:],
                in0=cen[:][:, None, :].broadcast_to([128, 16, T]),
                in1=thrL1[:, :, None].broadcast_to([128, 16, T]),
                op=OP.is_gt)
            nc.vector.tensor_reduce(out=cnt16[:], in_=msk[:], axis=AX.X,
                                    op=OP.add)
            c1row = wpool.tile([1, 16], F32, tag="c1row")
            rowsum(c1row[:], cnt16[:], 16)
            cntr16 = wpool.tile([128, 16], F32, tag="cntr16")
            bcast_row(cntr16[:], c1row[:], 16)
            # lo = (#edges with count >= k) - 1   (edges j = 0..15)
            ge16 = wpool.tile([128, 16], F32, tag="ge16")
            lo1 = wpool.tile([128, 1], F32, tag="lo1")
            nc.vector.tensor_scalar(out=ge16[:], in0=cntr16[:],
                                    scalar1=k3b[:], scalar2=None,
                                    op0=OP.is_ge, op1=OP.add,
                                    accum_out=lo1[:])
            nc.vector.tensor_scalar(out=lo1[:], in0=lo1[:], scalar1=-1.0,
                                    scalar2=None, op0=OP.add)
            lop1 = wpool.tile([128, 1], F32, tag="lop1")
            nc.gpsimd.tensor_scalar(out=lop1[:], in0=lo1[:], scalar1=1.0 / 16,
                                    scalar2=None, op0=OP.add)
            # level 2: thresholds lo + m/16 (io15 has (1..15)/16 then +999)
            thr2 = wpool.tile([128, 16], F32, tag="thr2")
            nc.vector.tensor_scalar(out=thr2[:], in0=io15,
                                    scalar1=lo1[:], scalar2=None,
                                    op0=OP.add)
            msc2 = msk
            c2 = wpool.tile([128, 16], F32, tag="c2")
            nc.vector.tensor_tensor(
                out=msc2[:],
                in0=cen[:][:, None, :].broadcast_to([128, 16, T]),
                in1=thr2[:][:, :, None].broadcast_to([128, 16, T]),
                op=OP.is_gt)
            nc.vector.tensor_reduce(out=c2[:], in_=msc2[:], axis=AX.X,
                                    op=OP.add)
            c2row = wpool.tile([1, 16], F32, tag="c2row")
            rowsum(c2row[:], c2[:], 16)
            c2r = wpool.tile([128, 16], F32, tag="c2r")
            bcast_row(c2r[:], c2row[:], 16)
            ge2 = wpool.tile([128, 16], F32, tag="ge2")
            mc = wpool.tile([128, 1], F32, tag="mc")
            nc.vector.tensor_scalar(out=ge2[:], in0=c2r[:],
                                    scalar1=k3b[:], scalar2=None,
                                    op0=OP.is_ge, op1=OP.add, accum_out=mc[:])
            hi1 = wpool.tile([128, 1], F32, tag="hi1")
            nc.vector.tensor_scalar(out=hi1[:], in0=mc[:],
                                    scalar1=1.0 / 16, scalar2=lop1[:],
                                    op0=OP.mult, op1=OP.add)
            # F(hi), count(hi), boundary max
            fsc = wpool.tile([128, T], F32, tag="fsc")
            fsa = wpool.tile([128, 1], F32, tag="fsa")
            nc.vector.scalar_tensor_tensor(
                out=fsc[:], in0=cen[:], scalar=hi1[:],
                in1=cen[:], op0=OP.is_gt, op1=OP.mult,
                accum_out=fsa[:])
            nc.scalar.copy(scadd[:, i, 0:1], fsa[:])
            cna = wpool.tile([128, 1], F32, tag="cna")
            nc.vector.tensor_scalar(out=fsc[:], in0=cen[:],
                                    scalar1=hi1[:], scalar2=None,
                                    op0=OP.is_gt, op1=OP.add, accum_out=cna[:])
            nc.scalar.copy(scadd[:, i, 1:2], cna[:])
            nc.vector.scalar_tensor_tensor(
                out=fsc[:], in0=cen[:], scalar=hi1[:],
                in1=cen[:], op0=OP.is_le, op1=OP.mult)
            bmt = wpool.tile([128, 1], F32, tag="bmt")
            nc.vector.tensor_reduce(out=bmt[:], in_=fsc[:], axis=AX.X, op=OP.max)
            nc.scalar.copy(bm4[:, i:i + 1], bmt[:])

        # ---------------- final combine (partition 0) ----------------
        rowsum(scrow[:].rearrange("p i s -> p (i s)"),
               scadd[:].rearrange("p i s -> p (i s)"), I * 4)
        maxreduce_row(bmrow[:], bm4[:], I)
        rowsum(uf4[:], ufall[:], I)

        k34r = bpool.tile([1, I], F32, tag="k34r")
        nc.vector.tensor_scalar(out=k34r[:], in0=nprow[:], scalar1=3.0,
                                scalar2=None, op0=OP.mult)
        r4 = bpool.tile([1, I], F32, tag="r4")
        nc.vector.tensor_sub(r4[:], k34r[:], scrow[:, :, 1])
        nc.vector.tensor_mul(r4[:], r4[:], bmrow[:])
        nc.vector.tensor_add(r4[:], r4[:], scrow[:, :, 0])   # mine sums
        cep = bpool.tile([1, I], F32, tag="cep")
        nc.vector.tensor_sub(cep[:], scrow[:, :, 2], uf4[:])  # ce_pos sums
        nc.vector.tensor_copy(out_sb[:, 0:4], nprow[:])
        nc.vector.tensor_copy(out_sb[:, 4:8], scrow[:, :, 3])
        nc.vector.tensor_copy(out_sb[:, 8:12], cep[:])
        nc.vector.tensor_copy(out_sb[:, 12:16], r4[:])
        nc.sync.dma_start(out=d_out[:, :], in_=out_sb[:])

    if fixup:
        _fixup_module(nc)
    return nc


def prepare_inputs(predicted_locs, predicted_scores, boxes, labels,
                   priors_centers):
    """Shard + marshal the full inputs into 8 per-core in_maps.

    All DRAM layouts are per-partition contiguous (partition-major), so
    every SBUF partition reads one contiguous chunk per DMA.
    """
    predicted_locs = np.asarray(predicted_locs, np.float32)
    predicted_scores = np.asarray(predicted_scores, np.float32)
    boxes = np.asarray(boxes, np.float32)
    labels_f = np.asarray(labels).astype(np.int64)
    priors = np.asarray(priors_centers, np.float32)

    npad = PP - P
    # scores: pad rows have class0=0, others -50 -> lse=0, S0=0, ce0=0 exactly
    pad_scores = np.full((B, npad, C), -50.0, np.float32)
    pad_scores[:, :, 0] = 0.0
    scores_p = np.concatenate([predicted_scores, pad_scores], axis=1)
    # [B, PP, C] -> [B, T, 128, C] -> [B, 128, T, C] -> [B, 128, T*C]
    scores_pm = scores_p.reshape(B, T, 128, C).transpose(0, 2, 1, 3)

    bx1, by1, bx2, by2 = (boxes[:, :, d] for d in range(4))
    barea = (bx2 - bx1) * (by2 - by1)
    q5 = np.stack([
        (bx1 + bx2) / 2, (by1 + by2) / 2,
        5.0 * np.log(bx2 - bx1), 5.0 * np.log(by2 - by1),
        np.zeros_like(bx1),
    ], axis=2).astype(np.float32)                           # [B, K, 5]
    qblk = np.zeros((B, 128, 8 * NQ), np.float32)
    for tb in range(8):
        qblk[:, tb * K:(tb + 1) * K, tb * NQ:(tb + 1) * NQ] = q5

    sco = np.zeros((B, 128, SW), np.float32)
    sco[:, :, :SO_QB] = scores_pm.reshape(B, 128, T * C)
    sco[:, :, SO_QB:] = qblk
    sco = _to_bf16(sco)

    # image pack: locs (t-major, tail zero) + broadcast box rows + label 1-hot
    ipack = np.zeros((B, 128, IW), np.float32)
    pcx0, pcy0, pw0, ph0 = (np.asarray(priors_centers, np.float32)[:, d]
                            for d in range(4))
    pofs = np.stack([pcx0 * (10.0 / pw0), pcy0 * (10.0 / ph0),
                     5.0 * np.log(pw0), 5.0 * np.log(ph0)], axis=1)  # [P, 4]
    locs_full = np.concatenate(
        [predicted_locs + pofs[None, :, :],
         np.zeros((B, npad, 4), np.float32)], axis=1)
    ipack[:, :, IO_LOC:IO_LOC + T * 4] = (
        locs_full.reshape(B, T, 128, 4).transpose(0, 2, 1, 3)
        .reshape(B, 128, T * 4))
    boxf = np.stack([bx1, by1, bx2, by2, barea], axis=1)    # [B, 5, K]
    ipack[:, :, IO_BB:IO_BB + 5 * K] = boxf.reshape(B, 1, 5 * K)
    lmask = (np.arange(C)[None, :, None] == labels_f[:, None, :])
    ipack[:, :C, IO_LM:IO_LM + K] = lmask.astype(np.float32)

    # const pack
    pad_pri = np.tile(np.array([-100.0, -100.0, 1.0, 1.0], np.float32),
                      (npad, 1))
    pri = np.concatenate([priors, pad_pri], axis=0)
    pcx, pcy, pw, ph = pri[:, 0], pri[:, 1], pri[:, 2], pri[:, 3]
    ptab = np.stack([
        pcx - pw / 2, pcy - ph / 2, pcx + pw / 2, pcy + ph / 2,
        pw * ph,
        pcx * (10.0 / pw), pcy * (10.0 / ph),
        10.0 / pw, 10.0 / ph,
        5.0 * np.log(pw), 5.0 * np.log(ph),
    ]).astype(np.float32)                                   # [11, PP]
    cst = np.zeros((128, CW), np.float32)
    # [11, PP] -> [11, T, 128] -> [128, 11, T]
    cst[:, CO_PT:CO_PT + 11 * T] = (
        ptab.reshape(11, T, 128).transpose(2, 0, 1).reshape(128, 11 * T))
    cst[:, CO_IDF:CO_IDF + 128] = np.eye(128, dtype=np.float32)
    cst[:, CO_IO15:CO_IO15 + 16] = np.concatenate(
        [np.arange(1, 16, dtype=np.float32) / 16.0, [999.0]])
    cst[:, CO_KV16:CO_KV16 + 16] = KV0 + KVS * np.arange(16, dtype=np.float32)
    cst[:, CO_THR:CO_THR + 16] = np.arange(16, dtype=np.float32)

    identb = _to_bf16(np.eye(128, dtype=np.float32))

    in_maps = []
    for c in range(NCORES):
        sl = slice(c * I, (c + 1) * I)
        in_maps.append({
            "sco": sco[sl],
            "ipack": np.ascontiguousarray(
                ipack[sl].transpose(1, 0, 2).reshape(128, I * IW)),
            "cst": cst,
            "identb": identb,
        })
    return in_maps


def combine_outputs(outs):
    """outs: list of 8 per-core [1,16] arrays -> scalar loss."""
    parts = np.concatenate([o.reshape(4, 4) for o in outs], axis=1)  # [4, 32]
    n_pos_total = parts[0].sum()
    box_sum = parts[1].sum()
    class_sum = parts[2].sum() + parts[3].sum()
    loss = class_sum / n_pos_total + box_sum / (n_pos_total * 4.0)
    return np.float32(loss)


_NC_CACHE = {}


def kernel(predicted_locs, predicted_scores, boxes, labels, priors_centers):
    if "nc" not in _NC_CACHE:
        _NC_CACHE["nc"] = build_nc()
    nc = _NC_CACHE["nc"]
    in_maps = prepare_inputs(predicted_locs, predicted_scores, boxes, labels,
                             priors_centers)
    res = run_bass_kernel_spmd(nc, in_maps, list(range(NCORES)))
    outs = [res.results[c]["out"] for c in range(NCORES)]
    return combine_outputs(outs)


if __name__ == "__main__":
    import reference as R

    inputs = {k: np.asarray(v) for k, v in R.setup_inputs().items()}
    print("loss =", kernel(**inputs))


# revision 23
# speedup vs baseline: 1.0592x; 1.0035x over previous
"""Trainium2 Bass kernel for SSD MultiBox loss (nn_ModelLoss_5970004541458).

Strategy: data-parallel over batch (32 images -> 8 cores x 4 images).
Per core, everything over the prior dim (P=8732, padded to 8960 = 70*128)
runs on-device:
  - jaccard matching in bf16 log-IoU space (monotone, so max/argmax/threshold
    comparisons are unchanged; threshold ln 0.5). Paired (x,y) ops halve the
    instruction count; bf16 doubles DVE throughput.
  - forced assignment via ADDITIVE sentinels ov + fmask*(100+4k): the 4-unit
    k spacing exceeds the ln-IoU range of forced points, so the largest k
    wins among colliding boxes (emulates the reference's last-wins scatter).
  - per-prior one-hot box gather via PE transpose + block-diag matmul (bf16)
  - CE: exp on ACT (bf16, 2 big chunks), class-sums on DVE (bf16),
    score-at-label via PE with sres stationary (81-col LDW, 16-col moving)
    and a CPU-precomputed label one-hot.
  - hard-negative mining via a 2-level 16-way counting grid with bounded-error
    boundary correction (no sort), per image.
All DRAM inputs are laid out per-partition-contiguous so every load is one
large DMA (128 descriptors of >=512B): one const pack, one image pack
(locs+boxes+labels-one-hot, CPU-pre-broadcast), one scores+qblk DMA per image.
Each core returns 16 partial sums; the host combines them into the loss.
"""
import sys

for _p in ("/opt/trn_rl_repo",):
    if _p not in sys.path:
        sys.path.insert(0, _p)

import numpy as np

import concourse.bass as bass
import concourse.tile as tile
from concourse import mybir
from concourse.bass_utils import run_bass_kernel_spmd

F32 = mybir.dt.float32
BF16 = mybir.dt.bfloat16
AX = mybir.AxisListType
OP = mybir.AluOpType
ACTF = mybir.ActivationFunctionType

B, P, C, K = 32, 8732, 81, 16
NCORES = 8
I = B // NCORES          # images per core = 4
PP = 8960                # padded priors = 70 * 128
T = PP // 128            # 70 prior tiles
T2 = 72                  # padded tile count for 128-col transpose blocks
NB = T2 * K // 128       # 9 transpose blocks of 128 (t,k)-columns
NCH = 2                  # score chunks per image (35 tiles each)
CT = T // NCH            # tiles per chunk = 35
THRESHOLD = 0.5
LN_THR = float(np.log(0.5))  # positives threshold in log-IoU space
KV0 = 100.0              # forced-assignment sentinel base (added to ln-IoU)
KVS = 4.0                # sentinel k spacing (> ln-IoU range of forced points)
NQ = 5                   # gathered quantities per box (cx, cy, 5lnw, 5lnh, pad)

# const pack column offsets (f32, [128, CW])
CO_PT = 0                # 11 prior-table rows x 70
CO_IDF = 770             # f32 identity 128
CO_IO15 = 898            # (1..15)/16 then 999
CO_KV16 = 914            # 100 + 4k
CO_THR = 930             # 0..15 level-1 mining thresholds
CW = 946

# image pack column offsets (f32, [128, I, IW])
IO_LOC = 0               # T2*4 locs (t-major, tail tiles zero)
IO_BB = 288              # 5x16 box rows (x1,y1,x2,y2,area), broadcast on CPU
IO_LM = 368              # label one-hot [81, 16] on partitions 0..80
IW = 384

# scores pack (bf16, [I, 128, SW])
SO_SC = 0                # 70*81 scores (t-major)
SO_QB = 5670             # 8*NQ block-diag gather stationary
SW = 5670 + 8 * NQ

_bf16 = np.dtype("uint16")  # bf16 carried as uint16 bit pattern if ml_dtypes absent
try:
    import ml_dtypes

    _bf16 = np.dtype(ml_dtypes.bfloat16)
except ImportError:
    ml_dtypes = None


def _to_bf16(x: np.ndarray) -> np.ndarray:
    if ml_dtypes is not None:
        return x.astype(ml_dtypes.bfloat16)
    u = x.astype(np.float32).view(np.uint32)
    rounded = ((u >> 16) + ((u >> 15) & 1)).astype(np.uint32)
    return (rounded & 0xFFFF).astype(np.uint16)


def _fixup_module(nc: bass.Bass) -> None:
    """Adapt the Tile-generated module to this container's walrus build.

    - EVENT_SEMAPHORE_RANGE_CLEAR is rejected ("ISA wrong length"); the
      preceding Drain(is_reset_sema) already resets the same range, so drop it.
    - Seq-only instructions accept fewer sync waits than Tile emits; hoist
      excess waits onto NoOps placed immediately before (same engine, so
      program order preserves semantics).
    """
    import bass_rust

    for f in nc.m.functions:
        for blk in f.blocks:
            newl = []
            for ins in blk.instructions:
                if getattr(ins, "op_name", None) == "EVENT_SEMAPHORE_RANGE_CLEAR":
                    continue
                si = ins.sync_info
                maxw = 1
                if si is not None and si.on_wait and len(si.on_wait) > maxw:
                    waits = list(si.on_wait)
                    extra, keep = waits[:-maxw], waits[-maxw:]
                    for j in range(0, len(extra), 1):
                        nop = mybir.InstNoOp(
                            name=f"{ins.name}-wsplit{j}", ins=[], outs=[],
                            engine=ins.engine)
                        nop.sync_info = bass_rust.SyncInfo(
                            on_wait=[extra[j]], on_update=[])
                        newl.append(nop)
                    ins.sync_info = bass_rust.SyncInfo(
                        on_wait=keep,
                        on_update=list(si.on_update) if si.on_update else [])
                newl.append(ins)
            blk.instructions = newl


def build_nc(fixup: bool = True) -> bass.Bass:
    nc = bass.Bass()

    d_sco = nc.dram_tensor("sco", [I, 128, SW], BF16, kind="ExternalInput")
    d_ipack = nc.dram_tensor("ipack", [128, I * IW], F32, kind="ExternalInput")
    d_cst = nc.dram_tensor("cst", [128, CW], F32, kind="ExternalInput")
    d_identb = nc.dram_tensor("identb", [128, 128], BF16, kind="ExternalInput")
    # out row layout (single partition): [np0..3, box0..3, cep0..3, mine0..3]
    d_out = nc.dram_tensor("out", [1, 16], F32, kind="ExternalOutput")

    from contextlib import ExitStack

    with tile.TileContext(nc) as tc, ExitStack() as es:
        cpool = es.enter_context(tc.tile_pool(name="consts", bufs=1))
        spool = es.enter_context(tc.tile_pool(name="scores", bufs=2))
        wpool = es.enter_context(tc.tile_pool(name="work", bufs=3))
        epool = es.enter_context(tc.tile_pool(name="exp", bufs=3))
        bpool = es.enter_context(tc.tile_pool(name="batched", bufs=1))
        pp_t = es.enter_context(tc.tile_pool(name="ps_t", bufs=1, space="PSUM"))
        pp_sel = es.enter_context(tc.tile_pool(name="ps_sel", bufs=1, space="PSUM"))
        pp_u = es.enter_context(tc.tile_pool(name="ps_u", bufs=1, space="PSUM"))
        pp_r = es.enter_context(tc.tile_pool(name="ps_r", bufs=2, space="PSUM"))

        # ---------------- constants (3 DMAs total) ----------------
        cpack = cpool.tile([128, CW], F32, tag="cpack")
        nc.sync.dma_start(out=cpack[:], in_=d_cst[:, :])
        ident = cpool.tile([128, 128], BF16, tag="ident")
        nc.sync.dma_start(out=ident[:], in_=d_identb[:, :])
        ipk = cpool.tile([128, I, IW], F32, tag="ipk")
        nc.sync.dma_start(out=ipk[:].rearrange("p i w -> p (i w)"),
                          in_=d_ipack[:, :])

        names = ["px1", "py1", "px2", "py2", "parea", "pcxn", "pcyn",
                 "ivx10", "ivy10", "lpw5", "lph5"]
        pt = {nm: cpack[:, CO_PT + r * T:CO_PT + (r + 1) * T]
              for r, nm in enumerate(names)}
        identf = cpack[:, CO_IDF:CO_IDF + 128]
        io15 = cpack[:, CO_IO15:CO_IO15 + 16]
        thrL1 = cpack[:, CO_THR:CO_THR + 16]

        ones_p = cpool.tile([128, 1], F32, tag="ones_p")
        nc.vector.memset(ones_p[:], 1.0)
        ones_r = cpool.tile([1, 128], F32, tag="ones_r")
        nc.vector.memset(ones_r[:], 1.0)
        ones_rb = cpool.tile([1, 128], BF16, tag="ones_rb")
        nc.vector.memset(ones_rb[:], 1.0)
        eps_b = cpool.tile([128, 1], F32, tag="eps_b")
        nc.vector.memset(eps_b[:], 1e-20)

        # bf16 staging copies of jaccard constants
        pt12b = cpool.tile([128, 2, T], BF16, tag="pt12b")
        nc.scalar.copy(pt12b[:].rearrange("p r t -> p (r t)"),
                       cpack[:, CO_PT:CO_PT + 2 * T])
        pt34b = cpool.tile([128, 2, T], BF16, tag="pt34b")
        nc.scalar.copy(pt34b[:].rearrange("p r t -> p (r t)"),
                       cpack[:, CO_PT + 2 * T:CO_PT + 4 * T])
        pareab = cpool.tile([128, T], BF16, tag="pareab")
        nc.scalar.copy(pareab[:], pt["parea"])
        kv1b = cpool.tile([128, 1], F32, tag="kv1b")
        nc.vector.memset(kv1b[:], KV0)

        def rowsum(dst_row_ap, src_ap, n):
            """[P, n] f32 -> [1, n] partition sum written to dst_row_ap."""
            ps = pp_r.tile([128, 128], F32, tag="red_bc")
            nc.tensor.matmul(ps[0:1, :n], lhsT=ones_p[:src_ap.shape[0], :],
                             rhs=src_ap, start=True, stop=True)
            nc.scalar.copy(dst_row_ap, ps[0:1, :n])

        def bcast_row(dst_ap, row_ap, n, bf=False):
            """[1, n] -> [128, n] replicated."""
            ps = pp_r.tile([128, 128], F32, tag="red_bc")
            nc.tensor.matmul(ps[:, :n], lhsT=ones_rb[:] if bf else ones_r[:],
                             rhs=row_ap, start=True, stop=True)
            nc.scalar.copy(dst_ap, ps[:, :n])

        def maxreduce_row(dst_row_ap, src_ap, n):
            """[128, n] f32 -> [1, n] partition max written to dst_row_ap."""
            ps = pp_r.tile([128, 128], F32, tag="red_bc")
            nc.tensor.transpose(ps[:n, :], src_ap, identf)
            tsb = wpool.tile([128, 128], F32, tag="red_tsb")
            nc.scalar.copy(tsb[:n, :], ps[:n, :])
            mx = wpool.tile([128, 1], F32, tag="red_mx")
            nc.vector.tensor_reduce(out=mx[:n, :], in_=tsb[:n, :],
                                    axis=AX.X, op=OP.max)
            ps2 = pp_r.tile([128, 128], F32, tag="red_bc")
            nc.tensor.transpose(ps2[0:1, :n], mx[:n, :], identf[:n, :n])
            nc.scalar.copy(dst_row_ap, ps2[0:1, :n])

        # IV4: [128, T2, 4] with d = (x, y, w, h); tail t>=T zeroed
        iv4 = cpool.tile([128, T2, 4], F32, tag="iv4")
        nc.vector.memset(iv4[:], 0.0)
        nc.vector.tensor_copy(iv4[:, :T, 0], pt["ivx10"])
        nc.vector.tensor_copy(iv4[:, :T, 1], pt["ivy10"])
        nc.vector.memset(iv4[:, :T, 2], 1.0)
        nc.vector.memset(iv4[:, :T, 3], 1.0)

        # persistent accumulators
        nprow = bpool.tile([1, I], F32, tag="nprow")
        scadd = bpool.tile([128, I, 4], F32, tag="scadd")   # fs, cn, lps, box
        scrow = bpool.tile([1, I, 4], F32, tag="scrow")
        bm4 = bpool.tile([128, I], F32, tag="bm4")
        bmrow = bpool.tile([1, I], F32, tag="bmrow")
        ufall = bpool.tile([C, I], F32, tag="ufall")
        uf4 = bpool.tile([1, I], F32, tag="uf4")
        out_sb = bpool.tile([1, 16], F32, tag="out_sb")

        for i in range(I):
            # ---------------- per-image load (1 DMA) ----------------
            sct = spool.tile([128, SW], BF16, tag="sct")
            nc.sync.dma_start(out=sct[:], in_=d_sco[i, :, :])
            sres = sct[:, :SO_QB].rearrange("p (t c) -> p t c", c=C)
            qblk = sct[:, SO_QB:SW]
            l4 = ipk[:, i, IO_LOC:IO_LOC + T2 * 4].rearrange(
                "p (t d) -> p t d", d=4)
            lmv = ipk[0:C, i, IO_LM:IO_LM + K]
            bbb = wpool.tile([128, 5, K], BF16, tag="bbb")
            nc.scalar.copy(bbb[:].rearrange("p a k -> p (a k)"),
                           ipk[:, i, IO_BB:IO_BB + 5 * K])

            # ---------------- jaccard, paired (x,y) in bf16 ----------------
            wh2 = wpool.tile([128, 2, T, K], BF16, tag="wh2")
            iu2 = wpool.tile([128, 2, T, K], BF16, tag="iu2")
            lnb = wpool.tile([128, 2, T, K], BF16, tag="lnb")
            ov = wpool.tile([128, T, K], BF16, tag="ov")
            nc.vector.tensor_tensor(
                out=iu2[:],
                in0=pt12b[:][:, :, :, None].broadcast_to([128, 2, T, K]),
                in1=bbb[:, 0:2, :][:, :, None, :].broadcast_to([128, 2, T, K]),
                op=OP.max)
            nc.vector.tensor_tensor(
                out=wh2[:],
                in0=pt34b[:][:, :, :, None].broadcast_to([128, 2, T, K]),
                in1=bbb[:, 2:4, :][:, :, None, :].broadcast_to([128, 2, T, K]),
                op=OP.min)
            nc.vector.tensor_sub(wh2[:], wh2[:], iu2[:])
            nc.scalar.activation(wh2[:], wh2[:], ACTF.Relu)
            nc.vector.tensor_mul(iu2[:, 0], wh2[:, 0], wh2[:, 1])
            nc.vector.tensor_tensor(
                out=iu2[:, 1],
                in0=pareab[:][:, :, None].broadcast_to([128, T, K]),
                in1=bbb[:, 4, :][:, None, :].broadcast_to([128, T, K]),
                op=OP.add)
            nc.vector.tensor_sub(iu2[:, 1], iu2[:, 1], iu2[:, 0])
            # log-space IoU: monotone, so comparisons unchanged
            nc.scalar.activation(lnb[:], iu2[:], ACTF.Ln, bias=eps_b[:])
            nc.vector.tensor_sub(ov[:], lnb[:, 0], lnb[:, 1])

            # ---------------- matching pass 2 ----------------
            m16 = wpool.tile([128, K], F32, tag="m16")
            m16r = wpool.tile([128, K], F32, tag="m16r")
            nc.vector.tensor_reduce(
                out=m16[:], in_=ov[:].rearrange("p t k -> p k t"),
                axis=AX.X, op=OP.max)
            m16row = wpool.tile([1, K], F32, tag="m16row")
            maxreduce_row(m16row[:], m16[:], K)
            bcast_row(m16r[:], m16row[:], K)
            fmask = wpool.tile([128, T, K], BF16, tag="fmask")
            nc.vector.tensor_tensor(
                out=fmask[:], in0=ov[:],
                in1=m16r[:][:, None, :].broadcast_to([128, T, K]),
                op=OP.is_equal)
            ovf = wpool.tile([128, T, K], BF16, tag="ovf")
            nc.vector.scalar_tensor_tensor(
                out=ovf[:], in0=fmask[:], scalar=kv1b[:], in1=ov[:],
                op0=OP.mult, op1=OP.add)
            pm = wpool.tile([128, T], BF16, tag="pm")
            nc.vector.tensor_reduce(out=pm[:], in_=ovf[:], axis=AX.X, op=OP.max)
            pos = wpool.tile([128, T], F32, tag="pos")
            npt = wpool.tile([128, 1], F32, tag="npt")
            nc.vector.tensor_scalar(out=pos[:], in0=pm[:],
                                    scalar1=LN_THR, scalar2=None,
                                    op0=OP.is_ge, op1=OP.add,
                                    accum_out=npt[:])
            # one-hot only for positives: negatives' pm*pos = (-)0 and their
            # ovf < ln(0.5) < 0 never equals it -> all-zero rows (gather and
            # U-matrix both want exactly that, so no separate wmat needed)
            pmm = wpool.tile([128, T], BF16, tag="pmm")
            nc.vector.tensor_mul(pmm[:], pm[:], pos[:])
            ohb = wpool.tile([128, T2 * K], BF16, tag="ohb")
            nc.vector.memset(ohb[:, T * K:], 0.0)
            nc.vector.tensor_tensor(
                out=ohb[:, :T * K].rearrange("p (t k) -> p t k", k=K),
                in0=ovf[:],
                in1=pmm[:][:, :, None].broadcast_to([128, T, K]),
                op=OP.is_equal)
            wmat = ohb[:, :T * K].rearrange("p (t k) -> p t k", k=K)

            # n_pos for this image
            rowsum(nprow[:, i:i + 1], npt[:], 1)
            npb = wpool.tile([128, 1], F32, tag="npb")
            bcast_row(npb[:], nprow[:, i:i + 1], 1)
            k3b = wpool.tile([128, 1], F32, tag="k3b")
            nc.gpsimd.tensor_scalar(out=k3b[:], in0=npb[:], scalar1=3.0,
                                    scalar2=None, op0=OP.mult)

            # ---------------- box gather via PE ----------------
            ohT_ps = pp_t.tile([128, NB, 128], BF16, tag="ohT")
            for b in range(NB):
                nc.tensor.transpose(
                    ohT_ps[:, b, :],
                    ohb[:, b * 128:(b + 1) * 128],
                    ident[:])
            ohT_sb = wpool.tile([128, NB * 128], BF16, tag="ohT_sb")
            nc.scalar.copy(ohT_sb[:], ohT_ps[:].rearrange("p b n -> p (b n)"))

            sel_ps = pp_sel.tile([8 * NQ, NB, 128], F32, tag="sel")
            for b in range(NB):
                nc.tensor.matmul(sel_ps[:, b, :], lhsT=qblk[:],
                                 rhs=ohT_sb[:, b * 128:(b + 1) * 128],
                                 start=True, stop=True)
            sel_sb = wpool.tile([8 * NQ, NB * 128], BF16, tag="sel_sb")
            nc.scalar.copy(sel_sb[:], sel_ps[:].rearrange("p b n -> p (b n)"))
            bk_ps = pp_t.tile([128, NB, 8 * NQ], BF16, tag="ohT")
            for b in range(NB):
                nc.tensor.transpose(
                    bk_ps[:, b, :],
                    sel_sb[:, b * 128:(b + 1) * 128],
                    ident[:8 * NQ, :8 * NQ])
            # bk_ps[p, (blk*40 + tb*5 + q)] = sel_q at t = blk*8+tb; read PSUM
            sel4 = (bk_ps[:].rearrange("p b n -> p (b n)")
                    .rearrange("p (t q) -> p t q", q=NQ)[:, :, 0:4])

            # ---------------- box L1 (l4 = locs + prior offsets, from CPU) ---
            tb1 = wpool.tile([128, T2, 4], F32, tag="tb1")
            nc.vector.tensor_mul(tb1[:], sel4, iv4[:])
            nc.vector.tensor_sub(tb1[:], l4, tb1[:])
            nc.vector.tensor_tensor(
                out=tb1[:, :T, :], in0=tb1[:, :T, :],
                in1=pos[:][:, :, None].broadcast_to([128, T, 4]),
                op=OP.mult)
            bacc = wpool.tile([128, 1], F32, tag="bacc")
            nc.scalar.activation(tb1[:], tb1[:], ACTF.Abs, accum_out=bacc[:])
            nc.scalar.copy(scadd[:, i, 3:4], bacc[:])

            # ------------- score at label: sres stationary on PE -------------
            u_ps = pp_u.tile([C, K], F32, tag="u")
            for t_ in range(T):
                nc.tensor.matmul(u_ps[:], lhsT=sres[:, t_, :],
                                 rhs=wmat[:, t_, :],
                                 start=(t_ == 0), stop=(t_ == T - 1))
            ufx = wpool.tile([C, K], F32, tag="ufx")
            ufa = wpool.tile([C, 1], F32, tag="ufa")
            nc.vector.tensor_mul(ufx[:], u_ps[:], lmv)
            nc.vector.tensor_scalar(out=ufx[:], in0=ufx[:], scalar1=1.0,
                                    scalar2=None, op0=OP.mult, op1=OP.add,
                                    accum_out=ufa[:])
            nc.scalar.copy(ufall[:, i:i + 1], ufa[:])

            # ---------------- CE: exp on ACT + DVE reduces ----------------
            se = wpool.tile([128, T], BF16, tag="se")
            for ch in range(NCH):
                et = epool.tile([128, CT, C], BF16, tag="exps")
                nc.scalar.activation(
                    et[:], sres[:, ch * CT:(ch + 1) * CT, :], ACTF.Exp)
                with nc.allow_low_precision("bf16 lse; 2e-2 loss tolerance"):
                    nc.vector.tensor_reduce(
                        out=se[:, ch * CT:(ch + 1) * CT],
                        in_=et[:], axis=AX.X, op=OP.add)

            lse = wpool.tile([128, T], F32, tag="lse")
            nc.scalar.activation(lse[:], se[:], ACTF.Ln)
            ce0 = wpool.tile([128, T], F32, tag="ce0")
            nc.vector.tensor_sub(ce0[:], lse[:], sres[:, :, 0])
            cen = wpool.tile([128, T], F32, tag="cen")
            nc.vector.scalar_tensor_tensor(
                out=cen[:], in0=pos[:], scalar=THRESHOLD, in1=ce0[:],
                op0=OP.is_lt, op1=OP.mult)
            # ce_pos partial: sum(lse * pos) (minus U part in final combine)
            lpst = wpool.tile([128, T], F32, tag="lpst")
            lps = wpool.tile([128, 1], F32, tag="lps")
            nc.vector.scalar_tensor_tensor(
                out=lpst[:], in0=pos[:], scalar=1.0, in1=lse[:],
                op0=OP.mult, op1=OP.mult, accum_out=lps[:])
            nc.scalar.copy(scadd[:, i, 2:3], lps[:])

            # ---------------- mining (2-level 16-way grid) ----------------
            msk = wpool.tile([128, 16, T], F32, tag="msk")
            cnt16 = wpool.tile([128, 16], F32, tag="cnt16")
            nc.vector.tensor_tensor(
                out=msk[:],
                in0=cen[:][:, None, :].broadcast_to([128, 16, T]),
                in1=thrL1[:, :, None].broadcast_to([128, 16, T]),
                op=OP.is_gt)
            nc.vector.tensor_reduce(out=cnt16[:], in_=msk[:], axis=AX.X,
                                    op=OP.add)
            c1row = wpool.tile([1, 16], F32, tag="c1row")
            rowsum(c1row[:], cnt16[:], 16)
            cntr16 = wpool.tile([128, 16], F32, tag="cntr16")
            bcast_row(cntr16[:], c1row[:], 16)
            # lo = (#edges with count >= k) - 1   (edges j = 0..15)
            ge16 = wpool.tile([128, 16], F32, tag="ge16")
            lo1 = wpool.tile([128, 1], F32, tag="lo1")
            nc.vector.tensor_scalar(out=ge16[:], in0=cntr16[:],
                                    scalar1=k3b[:], scalar2=None,
                                    op0=OP.is_ge, op1=OP.add,
                                    accum_out=lo1[:])
            nc.vector.tensor_scalar(out=lo1[:], in0=lo1[:], scalar1=-1.0,
                                    scalar2=None, op0=OP.add)
            lop1 = wpool.tile([128, 1], F32, tag="lop1")
            nc.gpsimd.tensor_scalar(out=lop1[:], in0=lo1[:], scalar1=1.0 / 16,
                                    scalar2=None, op0=OP.add)
            # level 2: thresholds lo + m/16 (io15 has (1..15)/16 then +999)
            thr2 = wpool.tile([128, 16], F32, tag="thr2")
            nc.vector.tensor_scalar(out=thr2[:], in0=io15,
                                    scalar1=lo1[:], scalar2=None,
                                    op0=OP.add)
            msc2 = msk
            c2 = wpool.tile([128, 16], F32, tag="c2")
            nc.vector.tensor_tensor(
                out=msc2[:],
                in0=cen[:][:, None, :].broadcast_to([128, 16, T]),
                in1=thr2[:][:, :, None].broadcast_to([128, 16, T]),
                op=OP.is_gt)
            nc.vector.tensor_reduce(out=c2[:], in_=msc2[:], axis=AX.X,
                                    op=OP.add)
            c2row = wpool.tile([1, 16], F32, tag="c2row")
            rowsum(c2row[:], c2[:], 16)
            c2r = wpool.tile([128, 16], F32, tag="c2r")
            bcast_row(c2r[:], c2row[:], 16)
            ge2 = wpool.tile([128, 16], F32, tag="ge2")
            mc = wpool.tile([128, 1], F32, tag="mc")
            nc.vector.tensor_scalar(out=ge2[:], in0=c2r[:],
                                    scalar1=k3b[:], scalar2=None,
                                    op0=OP.is_ge, op1=OP.add, accum_out=mc[:])
            hi1 = wpool.tile([128, 1], F32, tag="hi1")
            nc.vector.tensor_scalar(out=hi1[:], in0=mc[:],
                                    scalar1=1.0 / 16, scalar2=lop1[:],
                                    op0=OP.mult, op1=OP.add)
            # F(hi), count(hi), boundary max
            fsc = wpool.tile([128, T], F32, tag="fsc")
            fsa = wpool.tile([128, 1], F32, tag="fsa")
            nc.vector.scalar_tensor_tensor(
                out=fsc[:], in0=cen[:], scalar=hi1[:],
                in1=cen[:], op0=OP.is_gt, op1=OP.mult,
                accum_out=fsa[:])
            nc.scalar.copy(scadd[:, i, 0:1], fsa[:])
            cna = wpool.tile([128, 1], F32, tag="cna")
            nc.vector.tensor_scalar(out=fsc[:], in0=cen[:],
                                    scalar1=hi1[:], scalar2=None,
                                    op0=OP.is_gt, op1=OP.add, accum_out=cna[:])
            nc.scalar.copy(scadd[:, i, 1:2], cna[:])
            nc.vector.scalar_tensor_tensor(
                out=fsc[:], in0=cen[:], scalar=hi1[:],
                in1=cen[:], op0=OP.is_le, op1=OP.mult)
            bmt = wpool.tile([128, 1], F32, tag="bmt")
            nc.vector.tensor_reduce(out=bmt[:], in_=fsc[:], axis=AX.X, op=OP.max)
            nc.scalar.copy(bm4[:, i:i + 1], bmt[:])

        # ---------------- final combine (partition 0) ----------------
        rowsum(scrow[:].rearrange("p i s -> p (i s)"),
               scadd[:].rearrange("p i s -> p (i s)"), I * 4)
        maxreduce_row(bmrow[:], bm4[:], I)
        rowsum(uf4[:], ufall[:], I)

        k34r = bpool.tile([1, I], F32, tag="k34r")
        nc.vector.tensor_scalar(out=k34r[:], in0=nprow[:], scalar1=3.0,
                                scalar2=None, op0=OP.mult)
        r4 = bpool.tile([1, I], F32, tag="r4")
        nc.vector.tensor_sub(r4[:], k34r[:], scrow[:, :, 1])
        nc.vector.tensor_mul(r4[:], r4[:], bmrow[:])
        nc.vector.tensor_add(r4[:], r4[:], scrow[:, :, 0])   # mine sums
        cep = bpool.tile([1, I], F32, tag="cep")
        nc.vector.tensor_sub(cep[:], scrow[:, :, 2], uf4[:])  # ce_pos sums
        nc.vector.tensor_copy(out_sb[:, 0:4], nprow[:])
        nc.vector.tensor_copy(out_sb[:, 4:8], scrow[:, :, 3])
        nc.vector.tensor_copy(out_sb[:, 8:12], cep[:])
        nc.vector.tensor_copy(out_sb[:, 12:16], r4[:])
        nc.sync.dma_start(out=d_out[:, :], in_=out_sb[:])

    if fixup:
        _fixup_module(nc)
    return nc


def prepare_inputs(predicted_locs, predicted_scores, boxes, labels,
                   priors_centers):
    """Shard + marshal the full inputs into 8 per-core in_maps.

    All DRAM layouts are per-partition contiguous (partition-major), so
    every SBUF partition reads one contiguous chunk per DMA.
    """
    predicted_locs = np.asarray(predicted_locs, np.float32)
    predicted_scores = np.asarray(predicted_scores, np.float32)
    boxes = np.asarray(boxes, np.float32)
    labels_f = np.asarray(labels).astype(np.int64)
    priors = np.asarray(priors_centers, np.float32)

    npad = PP - P
    # scores: pad rows have class0=0, others -50 -> lse=0, S0=0, ce0=0 exactly
    pad_scores = np.full((B, npad, C), -50.0, np.float32)
    pad_scores[:, :, 0] = 0.0
    scores_p = np.concatenate([predicted_scores, pad_scores], axis=1)
    # [B, PP, C] -> [B, T, 128, C] -> [B, 128, T, C] -> [B, 128, T*C]
    scores_pm = scores_p.reshape(B, T, 128, C).transpose(0, 2, 1, 3)

    bx1, by1, bx2, by2 = (boxes[:, :, d] for d in range(4))
    barea = (bx2 - bx1) * (by2 - by1)
    q5 = np.stack([
        (bx1 + bx2) / 2, (by1 + by2) / 2,
        5.0 * np.log(bx2 - bx1), 5.0 * np.log(by2 - by1),
        np.zeros_like(bx1),
    ], axis=2).astype(np.float32)                           # [B, K, 5]
    qblk = np.zeros((B, 128, 8 * NQ), np.float32)
    for tb in range(8):
        qblk[:, tb * K:(tb + 1) * K, tb * NQ:(tb + 1) * NQ] = q5

    sco = np.zeros((B, 128, SW), np.float32)
    sco[:, :, :SO_QB] = scores_pm.reshape(B, 128, T * C)
    sco[:, :, SO_QB:] = qblk
    sco = _to_bf16(sco)

    # image pack: locs (t-major, tail zero) + broadcast box rows + label 1-hot
    ipack = np.zeros((B, 128, IW), np.float32)
    pcx0, pcy0, pw0, ph0 = (np.asarray(priors_centers, np.float32)[:, d]
                            for d in range(4))
    pofs = np.stack([pcx0 * (10.0 / pw0), pcy0 * (10.0 / ph0),
                     5.0 * np.log(pw0), 5.0 * np.log(ph0)], axis=1)  # [P, 4]
    locs_full = np.concatenate(
        [predicted_locs + pofs[None, :, :],
         np.zeros((B, npad, 4), np.float32)], axis=1)
    ipack[:, :, IO_LOC:IO_LOC + T * 4] = (
        locs_full.reshape(B, T, 128, 4).transpose(0, 2, 1, 3)
        .reshape(B, 128, T * 4))
    boxf = np.stack([bx1, by1, bx2, by2, barea], axis=1)    # [B, 5, K]
    ipack[:, :, IO_BB:IO_BB + 5 * K] = boxf.reshape(B, 1, 5 * K)
    lmask = (np.arange(C)[None, :, None] == labels_f[:, None, :])
    ipack[:, :C, IO_LM:IO_LM + K] = lmask.astype(np.float32)

    # const pack
    pad_pri = np.tile(np.array([-100.0, -100.0, 1.0, 1.0], np.float32),
                      (npad, 1))
    pri = np.concatenate([priors, pad_pri], axis=0)
    pcx, pcy, pw, ph = pri[:, 0], pri[:, 1], pri[:, 2], pri[:, 3]
    ptab = np.stack([
        pcx - pw / 2, pcy - ph / 2, pcx + pw / 2, pcy + ph / 2,
        pw * ph,
        pcx * (10.0 / pw), pcy * (10.0 / ph),
        10.0 / pw, 10.0 / ph,
        5.0 * np.log(pw), 5.0 * np.log(ph),
    ]).astype(np.float32)                                   # [11, PP]
    cst = np.zeros((128, CW), np.float32)
    # [11, PP] -> [11, T, 128] -> [128, 11, T]
    cst[:, CO_PT:CO_PT + 11 * T] = (
        ptab.reshape(11, T, 128).transpose(2, 0, 1).reshape(128, 11 * T))
    cst[:, CO_IDF:CO_IDF + 128] = np.eye(128, dtype=np.float32)
    cst[:, CO_IO15:CO_IO15 + 16] = np.concatenate(
        [np.arange(1, 16, dtype=np.float32) / 16.0, [999.0]])
    cst[:, CO_KV16:CO_KV16 + 16] = KV0 + KVS * np.arange(16, dtype=np.float32)
    cst[:, CO_THR:CO_THR + 16] = np.arange(16, dtype=np.float32)

    identb = _to_bf16(np.eye(128, dtype=np.float32))

    in_maps = []
    for c in range(NCORES):
        sl = slice(c * I, (c + 1) * I)
        in_maps.append({
            "sco": sco[sl],
            "ipack": np.ascontiguousarray(
                ipack[sl].transpose(1, 0, 2).reshape(128, I * IW)),
            "cst": cst,
            "identb": identb,
        })
    return in_maps


def combine_outputs(outs):
    """outs: list of 8 per-core [1,16] arrays -> scalar loss."""
    parts = np.concatenate([o.reshape(4, 4) for o in outs], axis=1)  # [4, 32]
    n_pos_total = parts[0].sum()
    box_sum = parts[1].sum()
    class_sum = parts[2].sum() + parts[3].sum()
    loss = class_sum / n_pos_total + box_sum / (n_pos_total * 4.0)
    return np.float32(loss)


_NC_CACHE = {}


def kernel(predicted_locs, predicted_scores, boxes, labels, priors_centers):
    if "nc" not in _NC_CACHE:
        _NC_CACHE["nc"] = build_nc()
    nc = _NC_CACHE["nc"]
    in_maps = prepare_inputs(predicted_locs, predicted_scores, boxes, labels,
                             priors_centers)
    res = run_bass_kernel_spmd(nc, in_maps, list(range(NCORES)))
    outs = [res.results[c]["out"] for c in range(NCORES)]
    return combine_outputs(outs)


if __name__ == "__main__":
    import reference as R

    inputs = {k: np.asarray(v) for k, v in R.setup_inputs().items()}
    print("loss =", kernel(**inputs))
